# revision 10
# baseline (speedup 1.0000x reference)
"""Multi-head self-attention (B=2, S=4096, D=512, H=8, Dh=64) on 8 TRN2 cores.

Sharding: core i handles batch b = i//4 and head-pair hp = i%4 (heads 2*hp,
2*hp+1).  Each core computes Q/K/V projections for its two heads, flash-style
attention (no-max softmax; scores range is +-6 so exp is safe), and a partial
out-projection.  Host sums the 4 partial outputs per batch and transposes back.

v2 design (ACT-exp-bound pipeline, ~2x over the f32r v1):
- All SBUF operands fp16: halves DMA + SBUF, enables fast weight load.
- Scores for the two heads (K=64 contraction each) are issued back-to-back so
  they land on PE row-groups (0,0)/(64,0) and run CONCURRENTLY (row packing).
- Exp is the bottleneck engine (ACT: 1 elem/cycle/lane @1.2GHz + ~300cyc/call).
  GPSIMD/Pool cannot read PSUM and neither can DMA, so every scores byte must
  exit PSUM through ACT or DVE; the ACT exp PSUM->SBUF(fp16) at [128, QB]
  granularity IS the optimal mover.  Everything else is kept off ACT.
- Softmax denominator: ones-column appended to V (col 64 of each 65-wide
  block); reciprocal via the 1-instruction reciprocal_approx_fast (DVE), then
  DRAM-bounce partition-broadcast, all off the critical path (per-head ctx
  accumulators; PSUM: 2x[128,QB] scores slots + 2x[65,QB] ctx accums = 8
  banks; phase A/C tiles rotate through the scores slots).
- Phases interleaved: projections emitted per 512-block with chunked input
  DMAs so attention starts ~2us in; out-projection per query-block right
  after normalization. ACT table prewarmed by a dummy exp at t=0.

TRN2 quirk: walrus encodes only ONE sync wait on TPB compute instructions.
`_legalize_matmul_waits` post-processes the scheduled module: extra waits move
onto injected single-wait same-engine no-ops placed directly before the
instruction in its block - semantically identical, walrus-legal.
"""

import sys
from contextlib import ExitStack

for _p in ("/opt/trn_rl_repo",):
    if _p not in sys.path:
        sys.path.insert(0, _p)

import numpy as np

import concourse.bass as bass
import concourse.tile as tile
from concourse import mybir
from concourse.bass_utils import run_bass_kernel_spmd

F32 = mybir.dt.float32
F16 = mybir.dt.float16
D = 512          # model dim
DH = 64          # head dim
P = 128          # partitions
B = 2
S_FULL = 4096
N_CORES = 8
NC_T = D // P    # 4 contraction tiles over model dim

LAST_RESULTS = None  # test harness reads exec_time_ns from here


def _emit(nc: bass.Bass, tc: "tile.TileContext", ctx: ExitStack, S: int):
    """Emit the per-core program. Parameterized by S for small-sim testing."""
    NBLK = S // 512          # 512-wide seq blocks
    NK = S // P              # 128-row key tiles
    QB = 1024 if S >= 1024 else S
    NQB = S // QB            # query blocks
    QH = QB // 512           # 512-wide halves per query block
    inv_scale = 1.0 / np.sqrt(DH)

    def mm(out, lhsT, rhs, start=True, stop=True):
        return nc.tensor.matmul(out, lhsT, rhs, start=start, stop=stop)

    xt = nc.declare_dram_parameter("xt", [D, S], F16, isOutput=False)
    wq = nc.declare_dram_parameter("wq", [D, P], F16, isOutput=False)
    wk = nc.declare_dram_parameter("wk", [D, P], F16, isOutput=False)
    wv = nc.declare_dram_parameter("wv", [D, P], F16, isOutput=False)
    wo = nc.declare_dram_parameter("wo", [P, D], F16, isOutput=False)
    yt = nc.declare_dram_parameter("yt", [D, S], F32, isOutput=True)

    const = ctx.enter_context(tc.tile_pool(name="const", bufs=1))

    # ---- weights straight to SBUF ----
    w_sb = {}
    for name, ap in (("wq", wq), ("wk", wk), ("wv", wv)):
        tiles = []
        for c in range(NC_T):
            t = const.tile([P, P], F16, tag=f"{name}{c}", name=f"{name}{c}")
            nc.sync.dma_start(out=t[:], in_=ap[c * P:(c + 1) * P, :])
            tiles.append(t)
        w_sb[name] = tiles
    wo_sb = const.tile([P, D], F16, tag="wo")
    nc.sync.dma_start(out=wo_sb[:], in_=wo[:, :])

    # xt loaded in per-block chunks (interleaved with phase A below)
    xt_sb = [const.tile([P, S], F16, tag=f"xt{c}", name=f"xt{c}")
             for c in range(NC_T)]

    # persistent intermediates
    qt_sb = const.tile([P, S], F16, tag="qt")      # [2*64 d, S] stacked heads
    kt_sb = const.tile([P, S], F16, tag="kt")
    # V with a ones column appended per k-tile: [128 k, NK*65]; col 64 == 1.0
    vones = [const.tile([P, NK * (DH + 1)], F16, tag=f"vones{h}", name=f"vones{h}")
             for h in range(2)]
    konst = const.tile([P, max(NK, 2), 1], F32, tag="konst")
    nc.vector.memset(konst[:], 1.0)
    for h in range(2):
        vv = vones[h].rearrange("p (k c) -> p k c", c=DH + 1)
        nc.vector.tensor_copy(vv[:, :, DH:DH + 1], konst[:, :NK, :])
    ctx_sb = const.tile([P, S], F16, tag="ctx")    # normalized context^T

    # ACT table prewarm: dummy exp so the ~2.7us table load overlaps phase A
    warm = const.tile([1, 2], F32, tag="warm")
    nc.scalar.activation(warm[:], konst[0:1, 0:2, 0], mybir.ActivationFunctionType.Exp)

    # PSUM: tag "s" 2x[128,QB] (2 banks each; also pq/pk/pv/o_ps) + 2 ctx accums
    ps = ctx.enter_context(tc.tile_pool(name="ps", bufs=2, space="PSUM"))
    ep = ctx.enter_context(tc.tile_pool(name="ep", bufs=2))
    bcp = ctx.enter_context(tc.tile_pool(name="bcp", bufs=2))
    rtp = ctx.enter_context(tc.tile_pool(name="rtp", bufs=2))
    rdp = ctx.enter_context(tc.tile_pool(name="rdp", bufs=2, space="DRAM"))
    osb = ctx.enter_context(tc.tile_pool(name="osb", bufs=2))

    # ---- phase A: projections, emitted per 512-block for early phase B start
    for blk in range(NBLK):
        sl = slice(blk * 512, (blk + 1) * 512)
        for c in range(NC_T):
            nc.sync.dma_start(out=xt_sb[c][:, sl], in_=xt[c * P:(c + 1) * P, sl])
        pk = ps.tile([P, 512], F32, tag="s", name="pk")
        for c in range(NC_T):
            mm(pk[:], w_sb["wk"][c][:], xt_sb[c][:, sl],
               start=(c == 0), stop=(c == NC_T - 1))
        nc.vector.tensor_copy(kt_sb[:, sl], pk[:])
        pq = ps.tile([P, 512], F32, tag="s", name="pq")
        for c in range(NC_T):
            mm(pq[:], w_sb["wq"][c][:], xt_sb[c][:, sl],
               start=(c == 0), stop=(c == NC_T - 1))
        nc.vector.tensor_copy(qt_sb[:, sl], pq[:])
        for k in range(4 * blk, min(4 * blk + 4, NK)):
            ksl = slice(k * P, (k + 1) * P)
            pv = ps.tile([P, P], F32, tag="s", name="pv")
            for c in range(NC_T):
                mm(pv[:], xt_sb[c][:, ksl], w_sb["wv"][c][:],
                   start=(c == 0), stop=(c == NC_T - 1))
            for h in range(2):
                nc.vector.tensor_copy(
                    vones[h][:, k * (DH + 1):k * (DH + 1) + DH],
                    pv[:, h * DH:(h + 1) * DH])

    # ---- phase B: attention (flash, no-max softmax) + phase C per q-block
    for qb in range(NQB):
        qsl = slice(qb * QB, (qb + 1) * QB)
        ctx_ps = [ps.tile([DH + 1, QB], F32, tag=f"ctx{h}", bufs=1,
                          name=f"ctx_ps{h}") for h in range(2)]
        for k in range(NK):
            s_ = []
            for h in range(2):
                s_.append(ps.tile([P, QB], F32, tag="s", name=f"s{h}"))
            for j in range(QH):  # adjacent h issue => PE row-packing
                jsl = slice(qb * QB + j * 512, qb * QB + (j + 1) * 512)
                for h in range(2):
                    hsl = slice(h * DH, (h + 1) * DH)
                    mm(s_[h][:, j * 512:(j + 1) * 512],
                       kt_sb[hsl, k * P:(k + 1) * P], qt_sb[hsl, jsl])
            for h in range(2):
                e = ep.tile([P, QB], F16, tag=f"e{h}", name=f"e{h}")
                nc.scalar.activation(e[:], s_[h][:],
                                     mybir.ActivationFunctionType.Exp,
                                     scale=inv_scale)
                vo = vones[h][:, k * (DH + 1):(k + 1) * (DH + 1)]
                for j in range(QH):
                    mm(ctx_ps[h][:, j * 512:(j + 1) * 512], vo,
                       e[:, j * 512:(j + 1) * 512],
                       start=(k == 0), stop=(k == NK - 1))
        # normalize: 1/denom (fast approx), partition-broadcast via DRAM bounce
        for h in range(2):
            hsl = slice(h * DH, (h + 1) * DH)
            rt = rtp.tile([1, QB], F32, tag="rt", name="rt")
            nc.vector.reciprocal(rt[0:1, :], ctx_ps[h][DH:DH + 1, :])
            rtd = rdp.tile([1, QB], F32, tag="rtd", name="rtd")
            nc.sync.dma_start(out=rtd[:], in_=rt[0:1, :])
            rtd_bcast = bass.AP(tensor=rtd.tensor, offset=rtd.offset,
                                ap=[[0, DH]] + list(rtd[0:1, :].ap)[1:])
            bc = bcp.tile([DH, QB], F32, tag="bc", name="bc")
            nc.sync.dma_start(out=bc[:], in_=rtd_bcast)
            nc.vector.tensor_mul(ctx_sb[hsl, qsl], ctx_ps[h][:DH, :], bc[:])
        # phase C: partial out-projection for this q-block
        for e4 in range(NC_T):
            for j in range(QH):
                jsl = slice(qb * QB + j * 512, qb * QB + (j + 1) * 512)
                o_ps = ps.tile([P, 512], F32, tag="s", name="o_ps")
                mm(o_ps[:], wo_sb[:, e4 * P:(e4 + 1) * P], ctx_sb[:, jsl])
                o_sb = osb.tile([P, 512], F32, tag="osb", name="o_sb")
                nc.vector.tensor_copy(o_sb[:], o_ps[:])
                nc.sync.dma_start(out=yt[e4 * P:(e4 + 1) * P, jsl], in_=o_sb[:])


_TPB_ENGINES = {mybir.EngineType.PE, mybir.EngineType.Activation,
                mybir.EngineType.DVE, mybir.EngineType.Pool}


def _legalize_matmul_waits(nc: bass.Bass) -> int:
    """Walrus encodes only ONE sync wait on TPB compute instructions (seen on
    Matmult and TensorCopy).  Move extra waits onto injected same-engine
    no-ops (one wait each) placed immediately before the instruction in its
    block: same semantics, legal encoding."""
    n_fixed = 0
    for f in nc.m.functions:
        for bb in f.blocks:
            out = []
            changed = False
            for ins in bb.instructions:
                si = ins.sync_info
                if (getattr(ins, "engine", None) is not None
                        and si is not None and len(si.on_wait) > 1):
                    for idx, w in enumerate(si.on_wait[:-1]):
                        nop = mybir.InstNoOp(name=f"{ins.name}-lgw{idx}",
                                             ins=[], outs=[])
                        nop.engine = ins.engine
                        nop.sync_info = mybir.SyncInfo(on_wait=[w], on_update=[])
                        out.append(nop)
                    ins.sync_info = mybir.SyncInfo(on_wait=[si.on_wait[-1]],
                                                   on_update=si.on_update)
                    n_fixed += 1
                    changed = True
                out.append(ins)
            if changed:
                bb.instructions = out
    return n_fixed


def build(S: int = S_FULL, legalize: bool = False) -> bass.Bass:
    nc = bass.Bass()
    with ExitStack() as ctx:
        ctx.enter_context(nc.allow_low_precision(
            reason="fp16 matmul operands / fp16 staged scores"))
        tc = ctx.enter_context(tile.TileContext(nc))
        _emit(nc, tc, ctx, S)
    if legalize:
        # only for the walrus/hardware path; CoreSim wants updates on every
        # instruction and doesn't enforce the 1-wait Matmult limit
        _legalize_matmul_waits(nc)
    return nc


_NC_CACHE = {}


def _get_nc(S: int) -> bass.Bass:
    if S not in _NC_CACHE:
        _NC_CACHE[S] = build(S, legalize=True)
    return _NC_CACHE[S]


def make_in_maps(X, Wq, Wk, Wv, Wo):
    xts = [np.ascontiguousarray(X[b].T).astype(np.float16) for b in range(B)]
    in_maps = []
    for i in range(N_CORES):
        b, hp = divmod(i, 4)  # 4 head-pairs per batch
        csl = slice(hp * P, (hp + 1) * P)
        in_maps.append({
            "xt": xts[b],
            "wq": np.ascontiguousarray(Wq[:, csl]).astype(np.float16),
            "wk": np.ascontiguousarray(Wk[:, csl]).astype(np.float16),
            "wv": np.ascontiguousarray(Wv[:, csl]).astype(np.float16),
            "wo": np.ascontiguousarray(Wo[csl, :]).astype(np.float16),
        })
    return in_maps


def kernel(X, Wq, Wk, Wv, Wo, _trace=False):
    global LAST_RESULTS
    X = np.asarray(X, dtype=np.float32)
    S = X.shape[1]
    nc = _get_nc(S)
    in_maps = make_in_maps(X, np.asarray(Wq, np.float32), np.asarray(Wk, np.float32),
                           np.asarray(Wv, np.float32), np.asarray(Wo, np.float32))
    res = run_bass_kernel_spmd(nc, in_maps, list(range(N_CORES)), trace=_trace)
    LAST_RESULTS = res
    Y = np.zeros((B, S, D), dtype=np.float32)
    for i in range(N_CORES):
        Y[i // 4] += res.results[i]["yt"].T
    return Y


# revision 13
# speedup vs baseline: 1.3652x; 1.3652x over previous
"""Multi-head self-attention (B=2, S=4096, D=512, H=8, Dh=64) on 8 TRN2 cores.

Sharding: core i handles batch b = i//4 and head-pair hp = i%4 (heads 2*hp,
2*hp+1).  Each core computes Q/K/V projections for its two heads, flash-style
attention (no-max softmax; scores range is +-6 so exp is safe), and a partial
out-projection.  Host sums the 4 partial outputs per batch and transposes back.

v3 design notes (ACT-exp-bound pipeline):
- All matmul operands bf16 (fp16 moving operands stream at 2 cycles/col on the
  PE - measured 426ns for N=512 - while bf16 runs 1 col/cycle).
- Exp is the bottleneck engine (ACT: 1 elem/cycle/lane @1.2GHz + ~350cyc/call
  => 256 x 1147ns = 294us/core).  GPSIMD/Pool and DMA cannot read PSUM, so
  every scores element must leave PSUM through ACT or DVE; the ACT exp
  PSUM->SBUF(bf16) at [128, QB] granularity IS the optimal mover.
- Engine queues execute IN ORDER, so overlap is an emission-order problem:
  * ctx matmuls for k are emitted after the scores matmuls for k+1
    (software pipelining) - otherwise ctx(k), which waits on exp(k), blocks
    the already-runnable scores(k+1) in the PE queue and ACT starves.
  * projection blocks (phase A) are emitted interleaved into the first
    query-block's k-loop, just ahead of the k-tiles that consume them.
  * the out-projection for query-block qb is emitted in small pieces inside
    qb+1's k-loop, after the normalization data is long since ready.
- Normalization: ones-column appended to V gives the denominator row; the ctx
  accumulator is drained PSUM->SBUF by one DVE copy (freeing the PSUM bank in
  ~1.2us), the denominator row is partition-broadcast via a DRAM bounce
  (stride-0 partition APs are legal on DRAM), and the divide runs on the
  otherwise-idle GPSIMD/Pool engine, all off the critical path.
- PSUM: 2 x [128,QB] scores slots + 2 x [65,QB] ctx accumulators = 8 banks;
  phase A/C tiles rotate through the scores slots.
- ACT table prewarmed by a dummy exp at t=0 so the ~2.7us load overlaps DMA.

TRN2 quirk: walrus encodes only ONE sync wait on TPB compute instructions.
`_legalize_matmul_waits` post-processes the scheduled module: extra waits move
onto injected single-wait same-engine no-ops placed directly before the
instruction in its block - semantically identical, walrus-legal.
"""

import sys
from contextlib import ExitStack

for _p in ("/opt/trn_rl_repo",):
    if _p not in sys.path:
        sys.path.insert(0, _p)

import ml_dtypes
import numpy as np

import concourse.bass as bass
import concourse.tile as tile
from concourse import mybir
from concourse.bass_utils import run_bass_kernel_spmd

F32 = mybir.dt.float32
BF16 = mybir.dt.bfloat16
NP_BF16 = ml_dtypes.bfloat16
D = 512          # model dim
DH = 64          # head dim
P = 128          # partitions
B = 2
S_FULL = 4096
N_CORES = 8
NC_T = D // P    # 4 contraction tiles over model dim

LAST_RESULTS = None  # test harness reads exec_time_ns from here


def _emit(nc: bass.Bass, tc: "tile.TileContext", ctx: ExitStack, S: int):
    """Emit the per-core program. Parameterized by S for small-sim testing."""
    NBLK = S // 512          # 512-wide seq blocks
    NK = S // P              # 128-row key tiles
    QB = 1024 if S >= 1024 else S
    NQB = S // QB            # query blocks
    QH = QB // 512           # 512-wide halves per query block
    inv_scale = 1.0 / np.sqrt(DH)

    def mm(out, lhsT, rhs, start=True, stop=True):
        return nc.tensor.matmul(out, lhsT, rhs, start=start, stop=stop)

    xt = nc.declare_dram_parameter("xt", [D, S], BF16, isOutput=False)
    wq = nc.declare_dram_parameter("wq", [D, P], BF16, isOutput=False)
    wk = nc.declare_dram_parameter("wk", [D, P], BF16, isOutput=False)
    wv = nc.declare_dram_parameter("wv", [D, P], BF16, isOutput=False)
    wo = nc.declare_dram_parameter("wo", [P, D], BF16, isOutput=False)
    yt = nc.declare_dram_parameter("yt", [D, S], F32, isOutput=True)

    const = ctx.enter_context(tc.tile_pool(name="const", bufs=1))

    # ---- weights straight to SBUF ----
    w_sb = {}
    for name, ap in (("wq", wq), ("wk", wk), ("wv", wv)):
        tiles = []
        for c in range(NC_T):
            t = const.tile([P, P], BF16, tag=f"{name}{c}", name=f"{name}{c}")
            nc.sync.dma_start(out=t[:], in_=ap[c * P:(c + 1) * P, :])
            tiles.append(t)
        w_sb[name] = tiles
    wo_sb = const.tile([P, D], BF16, tag="wo")
    nc.sync.dma_start(out=wo_sb[:], in_=wo[:, :])

    # xt loaded in per-block chunks (interleaved with phase A below)
    xt_sb = [const.tile([P, S], BF16, tag=f"xt{c}", name=f"xt{c}")
             for c in range(NC_T)]

    # persistent intermediates
    qt_sb = const.tile([P, S], BF16, tag="qt")      # [2*64 d, S] stacked heads
    kt_sb = const.tile([P, S], BF16, tag="kt")
    # V with a ones column appended per k-tile: [128 k, NK*65]; col 64 == 1.0
    vones = [const.tile([P, NK * (DH + 1)], BF16, tag=f"vones{h}", name=f"vones{h}")
             for h in range(2)]
    konst = const.tile([P, max(NK, 2), 1], F32, tag="konst")
    nc.vector.memset(konst[:], 1.0)
    for h in range(2):
        vv = vones[h].rearrange("p (k c) -> p k c", c=DH + 1)
        nc.vector.tensor_copy(vv[:, :, DH:DH + 1], konst[:, :NK, :])
    ctx_sb = const.tile([P, S], BF16, tag="ctx")    # normalized context^T

    # ACT table prewarm: dummy exp so the ~2.7us table load overlaps phase A
    warm = const.tile([1, 2], F32, tag="warm")
    nc.scalar.activation(warm[:], konst[0:1, 0:2, 0], mybir.ActivationFunctionType.Exp)

    # PSUM: tag "s" 2x[128,QB] (2 banks each; also pq/pk/pv/o_ps) + 2 ctx accums
    ps = ctx.enter_context(tc.tile_pool(name="ps", bufs=2, space="PSUM"))
    ep = ctx.enter_context(tc.tile_pool(name="ep", bufs=3))
    bcp = ctx.enter_context(tc.tile_pool(name="bcp", bufs=2))
    cdp = ctx.enter_context(tc.tile_pool(name="cdp", bufs=2))
    rdp = ctx.enter_context(tc.tile_pool(name="rdp", bufs=2, space="DRAM"))
    osb = ctx.enter_context(tc.tile_pool(name="osb", bufs=2))

    def emit_a(blk):
        """Projections for one 512-wide block: xt DMA, K, Q, V(+ones)."""
        sl = slice(blk * 512, (blk + 1) * 512)
        for c in range(NC_T):
            nc.sync.dma_start(out=xt_sb[c][:, sl], in_=xt[c * P:(c + 1) * P, sl])
        pk = ps.tile([P, 512], F32, tag="s", name="pk")
        for c in range(NC_T):
            mm(pk[:], w_sb["wk"][c][:], xt_sb[c][:, sl],
               start=(c == 0), stop=(c == NC_T - 1))
        nc.vector.tensor_copy(kt_sb[:, sl], pk[:])
        pq = ps.tile([P, 512], F32, tag="s", name="pq")
        for c in range(NC_T):
            mm(pq[:], w_sb["wq"][c][:], xt_sb[c][:, sl],
               start=(c == 0), stop=(c == NC_T - 1))
        nc.vector.tensor_copy(qt_sb[:, sl], pq[:])
        for k in range(4 * blk, min(4 * blk + 4, NK)):
            ksl = slice(k * P, (k + 1) * P)
            pv = ps.tile([P, P], F32, tag="s", name="pv")
            for c in range(NC_T):
                mm(pv[:], xt_sb[c][:, ksl], w_sb["wv"][c][:],
                   start=(c == 0), stop=(c == NC_T - 1))
            for h in range(2):
                nc.vector.tensor_copy(
                    vones[h][:, k * (DH + 1):k * (DH + 1) + DH],
                    pv[:, h * DH:(h + 1) * DH])

    def emit_c(qb, piece):
        """One out-projection piece (of NC_T*QH) for query block qb."""
        e4, j = divmod(piece, QH)
        jsl = slice(qb * QB + j * 512, qb * QB + (j + 1) * 512)
        o_ps = ps.tile([P, 512], F32, tag="s", name="o_ps")
        mm(o_ps[:], wo_sb[:, e4 * P:(e4 + 1) * P], ctx_sb[:, jsl])
        o_sb = osb.tile([P, 512], F32, tag="osb", name="o_sb")
        nc.vector.tensor_copy(o_sb[:], o_ps[:])
        nc.sync.dma_start(out=yt[e4 * P:(e4 + 1) * P, jsl], in_=o_sb[:])

    # phase A lead-in: enough blocks for the first k-tiles; rest interleaved
    pre_blks = min(2, NBLK)
    for blk in range(pre_blks):
        emit_a(blk)

    NPIECE = NC_T * QH
    for qb in range(NQB):
        qsl = slice(qb * QB, (qb + 1) * QB)
        ctx_ps = [ps.tile([DH + 1, QB], F32, tag=f"ctx{h}", bufs=1,
                          name=f"ctx_ps{h}") for h in range(2)]
        pend = None  # (k, [e_h0, e_h1]) whose ctx matmuls are not yet emitted

        def emit_ctx(pk, pe):
            for h in range(2):
                vo = vones[h][:, pk * (DH + 1):(pk + 1) * (DH + 1)]
                for j in range(QH):
                    mm(ctx_ps[h][:, j * 512:(j + 1) * 512], vo,
                       pe[h][:, j * 512:(j + 1) * 512],
                       start=(pk == 0), stop=(pk == NK - 1))

        for k in range(NK):
            # interleave remaining phase A blocks 1.5 blocks ahead of use
            if qb == 0:
                blk = (k + 6) // 4
                if pre_blks <= blk < NBLK and (k + 6) % 4 == 0:
                    emit_a(blk)
            # interleave previous query block's out-projection
            if qb > 0 and 2 <= k < 2 + NPIECE:
                emit_c(qb - 1, k - 2)
            s_ = [ps.tile([P, QB], F32, tag="s", name=f"s{h}") for h in range(2)]
            for j in range(QH):  # adjacent h issue => PE row-group packing
                jsl = slice(qb * QB + j * 512, qb * QB + (j + 1) * 512)
                for h in range(2):
                    hsl = slice(h * DH, (h + 1) * DH)
                    mm(s_[h][:, j * 512:(j + 1) * 512],
                       kt_sb[hsl, k * P:(k + 1) * P], qt_sb[hsl, jsl])
            e_ = []
            for h in range(2):
                e = ep.tile([P, QB], BF16, tag=f"e{h}", name=f"e{h}")
                nc.scalar.activation(e[:], s_[h][:],
                                     mybir.ActivationFunctionType.Exp,
                                     scale=inv_scale)
                e_.append(e)
            if pend is not None:
                emit_ctx(*pend)
            pend = (k, e_)
        emit_ctx(*pend)

        # normalize: drain accumulator fast (frees PSUM), then divide on Pool
        for h in range(2):
            hsl = slice(h * DH, (h + 1) * DH)
            cd = cdp.tile([DH + 1, QB], F32, tag=f"cd{h}", name=f"cd{h}")
            nc.vector.tensor_copy(cd[:], ctx_ps[h][:])
            nc.vector.reciprocal(cd[DH:DH + 1, :], cd[DH:DH + 1, :])
            rtd = rdp.tile([1, QB], F32, tag="rtd", name="rtd")
            nc.sync.dma_start(out=rtd[:], in_=cd[DH:DH + 1, :])
            rtd_bcast = bass.AP(tensor=rtd.tensor, offset=rtd.offset,
                                ap=[[0, DH]] + list(rtd[0:1, :].ap)[1:])
            bc = bcp.tile([DH, QB], F32, tag="bc", name="bc")
            nc.sync.dma_start(out=bc[:], in_=rtd_bcast)
            nc.vector.tensor_mul(ctx_sb[hsl, qsl], cd[:DH, :], bc[:])

    # out-projection for the final query block
    for piece in range(NPIECE):
        emit_c(NQB - 1, piece)


_TPB_ENGINES = {mybir.EngineType.PE, mybir.EngineType.Activation,
                mybir.EngineType.DVE, mybir.EngineType.Pool}


def _legalize_matmul_waits(nc: bass.Bass) -> int:
    """Walrus encodes only ONE sync wait on TPB compute instructions (seen on
    Matmult and TensorCopy).  Move extra waits onto injected same-engine
    no-ops (one wait each) placed immediately before the instruction in its
    block: same semantics, legal encoding."""
    n_fixed = 0
    for f in nc.m.functions:
        for bb in f.blocks:
            out = []
            changed = False
            for ins in bb.instructions:
                si = ins.sync_info
                if (getattr(ins, "engine", None) is not None
                        and si is not None and len(si.on_wait) > 1):
                    for idx, w in enumerate(si.on_wait[:-1]):
                        nop = mybir.InstNoOp(name=f"{ins.name}-lgw{idx}",
                                             ins=[], outs=[])
                        nop.engine = ins.engine
                        nop.sync_info = mybir.SyncInfo(on_wait=[w], on_update=[])
                        out.append(nop)
                    ins.sync_info = mybir.SyncInfo(on_wait=[si.on_wait[-1]],
                                                   on_update=si.on_update)
                    n_fixed += 1
                    changed = True
                out.append(ins)
            if changed:
                bb.instructions = out
    return n_fixed


def build(S: int = S_FULL, legalize: bool = False) -> bass.Bass:
    nc = bass.Bass()
    with ExitStack() as ctx:
        ctx.enter_context(nc.allow_low_precision(
            reason="bf16 matmul operands / bf16 exp output"))
        tc = ctx.enter_context(tile.TileContext(nc))
        _emit(nc, tc, ctx, S)
    if legalize:
        # only for the walrus/hardware path; CoreSim wants updates on every
        # instruction and doesn't enforce the 1-wait Matmult limit
        _legalize_matmul_waits(nc)
    return nc


_NC_CACHE = {}


def _get_nc(S: int) -> bass.Bass:
    if S not in _NC_CACHE:
        _NC_CACHE[S] = build(S, legalize=True)
    return _NC_CACHE[S]


def make_in_maps(X, Wq, Wk, Wv, Wo):
    xts = [np.ascontiguousarray(X[b].T).astype(NP_BF16) for b in range(B)]
    in_maps = []
    for i in range(N_CORES):
        b, hp = divmod(i, 4)  # 4 head-pairs per batch
        csl = slice(hp * P, (hp + 1) * P)
        in_maps.append({
            "xt": xts[b],
            "wq": np.ascontiguousarray(Wq[:, csl]).astype(NP_BF16),
            "wk": np.ascontiguousarray(Wk[:, csl]).astype(NP_BF16),
            "wv": np.ascontiguousarray(Wv[:, csl]).astype(NP_BF16),
            "wo": np.ascontiguousarray(Wo[csl, :]).astype(NP_BF16),
        })
    return in_maps


def kernel(X, Wq, Wk, Wv, Wo, _trace=False):
    global LAST_RESULTS
    X = np.asarray(X, dtype=np.float32)
    S = X.shape[1]
    nc = _get_nc(S)
    in_maps = make_in_maps(X, np.asarray(Wq, np.float32), np.asarray(Wk, np.float32),
                           np.asarray(Wv, np.float32), np.asarray(Wo, np.float32))
    res = run_bass_kernel_spmd(nc, in_maps, list(range(N_CORES)), trace=_trace)
    LAST_RESULTS = res
    Y = np.zeros((B, S, D), dtype=np.float32)
    for i in range(N_CORES):
        Y[i // 4] += res.results[i]["yt"].T
    return Y


# revision 17
# speedup vs baseline: 1.5102x; 1.1062x over previous
"""Multi-head self-attention (B=2, S=4096, D=512, H=8, Dh=64) on 8 TRN2 cores.

Sharding: core i handles batch b = i//4 and head-pair hp = i%4 (heads 2*hp,
2*hp+1).  Each core computes Q/K/V projections for its two heads, flash-style
attention (no-max softmax; scores range is +-6 so exp is safe), and a partial
out-projection.  Host sums the 4 partial outputs per batch and transposes back.

v3 design notes (ACT-exp-bound pipeline):
- All matmul operands bf16 (fp16 moving operands stream at 2 cycles/col on the
  PE - measured 426ns for N=512 - while bf16 runs 1 col/cycle).
- Exp is the bottleneck engine (ACT: 1 elem/cycle/lane @1.2GHz + ~350cyc/call
  => 256 x 1147ns = 294us/core).  GPSIMD/Pool and DMA cannot read PSUM, so
  every scores element must leave PSUM through ACT or DVE; the ACT exp
  PSUM->SBUF(bf16) at [128, QB] granularity IS the optimal mover.
- Engine queues execute IN ORDER, so overlap is an emission-order problem:
  * ctx matmuls for k are emitted after the scores matmuls for k+1
    (software pipelining) - otherwise ctx(k), which waits on exp(k), blocks
    the already-runnable scores(k+1) in the PE queue and ACT starves.
  * projection blocks (phase A) are emitted interleaved into the first
    query-block's k-loop, just ahead of the k-tiles that consume them.
  * the out-projection for query-block qb is emitted in small pieces inside
    qb+1's k-loop, after the normalization data is long since ready.
- Normalization: ones-column appended to V gives the denominator row; the ctx
  accumulator is drained PSUM->SBUF by one DVE copy (freeing the PSUM bank in
  ~1.2us), the denominator row is partition-broadcast via a DRAM bounce
  (stride-0 partition APs are legal on DRAM), and the divide runs on the
  otherwise-idle GPSIMD/Pool engine, all off the critical path.
- PSUM: 2 x [128,QB] scores slots + 2 x [65,QB] ctx accumulators = 8 banks;
  phase A/C tiles rotate through the scores slots.
- ACT table prewarmed by a dummy exp at t=0 so the ~2.7us load overlaps DMA.

TRN2 quirk: walrus encodes only ONE sync wait on TPB compute instructions.
`_legalize_matmul_waits` post-processes the scheduled module: extra waits move
onto injected single-wait same-engine no-ops placed directly before the
instruction in its block - semantically identical, walrus-legal.
"""

import sys
from contextlib import ExitStack

for _p in ("/opt/trn_rl_repo",):
    if _p not in sys.path:
        sys.path.insert(0, _p)

import ml_dtypes
import numpy as np

import concourse.bass as bass
import concourse.tile as tile
from concourse import mybir
from concourse.bass_utils import run_bass_kernel_spmd

F32 = mybir.dt.float32
BF16 = mybir.dt.bfloat16
NP_BF16 = ml_dtypes.bfloat16
D = 512          # model dim
DH = 64          # head dim
P = 128          # partitions
B = 2
S_FULL = 4096
N_CORES = 8
NC_T = D // P    # 4 contraction tiles over model dim

LAST_RESULTS = None  # test harness reads exec_time_ns from here


def _emit(nc: bass.Bass, tc: "tile.TileContext", ctx: ExitStack, S: int):
    """Emit the per-core program. Parameterized by S for small-sim testing."""
    NBLK = S // 512          # 512-wide seq blocks
    NK = S // P              # 128-row key tiles
    QB = 1024 if S >= 1024 else S
    NQB = S // QB            # query blocks
    QH = QB // 512           # 512-wide halves per query block
    inv_scale = 1.0 / np.sqrt(DH)

    def mm(out, lhsT, rhs, start=True, stop=True):
        return nc.tensor.matmul(out, lhsT, rhs, start=start, stop=stop)

    xt = nc.declare_dram_parameter("xt", [D, S], BF16, isOutput=False)
    wq = nc.declare_dram_parameter("wq", [D, P], BF16, isOutput=False)
    wk = nc.declare_dram_parameter("wk", [D, P], BF16, isOutput=False)
    wv = nc.declare_dram_parameter("wv", [D, P], BF16, isOutput=False)
    wo = nc.declare_dram_parameter("wo", [P, D], BF16, isOutput=False)
    yt = nc.declare_dram_parameter("yt", [D, S], F32, isOutput=True)

    const = ctx.enter_context(tc.tile_pool(name="const", bufs=1))

    # ---- weights straight to SBUF ----
    w_sb = {}
    for name, ap in (("wq", wq), ("wk", wk), ("wv", wv)):
        tiles = []
        for c in range(NC_T):
            t = const.tile([P, P], BF16, tag=f"{name}{c}", name=f"{name}{c}")
            nc.sync.dma_start(out=t[:], in_=ap[c * P:(c + 1) * P, :])
            tiles.append(t)
        w_sb[name] = tiles
    wo_sb = const.tile([P, D], BF16, tag="wo")
    nc.sync.dma_start(out=wo_sb[:], in_=wo[:, :])

    # xt loaded in per-block chunks (interleaved with phase A below)
    xt_sb = [const.tile([P, S], BF16, tag=f"xt{c}", name=f"xt{c}")
             for c in range(NC_T)]

    # persistent intermediates
    qt_sb = const.tile([P, S], BF16, tag="qt")      # [2*64 d, S] stacked heads
    kt_sb = const.tile([P, S], BF16, tag="kt")
    # V with a ones column appended per k-tile: [128 k, NK*65]; col 64 == 1.0
    vones = [const.tile([P, NK * (DH + 1)], BF16, tag=f"vones{h}", name=f"vones{h}")
             for h in range(2)]
    konst = const.tile([P, max(NK, 2), 1], F32, tag="konst")
    nc.vector.memset(konst[:], 1.0)
    for h in range(2):
        vv = vones[h].rearrange("p (k c) -> p k c", c=DH + 1)
        nc.vector.tensor_copy(vv[:, :, DH:DH + 1], konst[:, :NK, :])
    ctx_sb = const.tile([P, S], BF16, tag="ctx")    # normalized context^T

    # ACT table prewarm: dummy exp so the ~2.7us table load overlaps phase A
    warm = const.tile([1, 2], F32, tag="warm")
    nc.scalar.activation(warm[:], konst[0:1, 0:2, 0], mybir.ActivationFunctionType.Exp)

    # PSUM: tag "s" 2x[128,QB] (2 banks each; also pq/pk/pv/o_ps) + 2 ctx accums
    ps = ctx.enter_context(tc.tile_pool(name="ps", bufs=2, space="PSUM"))
    ep = ctx.enter_context(tc.tile_pool(name="ep", bufs=3))
    bcp = ctx.enter_context(tc.tile_pool(name="bcp", bufs=2))
    cdp = ctx.enter_context(tc.tile_pool(name="cdp", bufs=2))
    rdp = ctx.enter_context(tc.tile_pool(name="rdp", bufs=2, space="DRAM"))
    osb = ctx.enter_context(tc.tile_pool(name="osb", bufs=2))

    def emit_xt_dma(blk):
        sl = slice(blk * 512, (blk + 1) * 512)
        for c in range(NC_T):
            nc.sync.dma_start(out=xt_sb[c][:, sl], in_=xt[c * P:(c + 1) * P, sl])

    def emit_kq(blk):
        """K and Q projections for one 512-wide block."""
        sl = slice(blk * 512, (blk + 1) * 512)
        pk = ps.tile([P, 512], F32, tag="s", name="pk")
        for c in range(NC_T):
            mm(pk[:], w_sb["wk"][c][:], xt_sb[c][:, sl],
               start=(c == 0), stop=(c == NC_T - 1))
        nc.vector.tensor_copy(kt_sb[:, sl], pk[:])
        pq = ps.tile([P, 512], F32, tag="s", name="pq")
        for c in range(NC_T):
            mm(pq[:], w_sb["wq"][c][:], xt_sb[c][:, sl],
               start=(c == 0), stop=(c == NC_T - 1))
        nc.vector.tensor_copy(qt_sb[:, sl], pq[:])

    def emit_v(blk):
        """V projection (+ones column layout) for one 512-wide block."""
        for k in range(4 * blk, min(4 * blk + 4, NK)):
            ksl = slice(k * P, (k + 1) * P)
            pv = ps.tile([P, P], F32, tag="s", name="pv")
            for c in range(NC_T):
                mm(pv[:], xt_sb[c][:, ksl], w_sb["wv"][c][:],
                   start=(c == 0), stop=(c == NC_T - 1))
            for h in range(2):
                nc.vector.tensor_copy(
                    vones[h][:, k * (DH + 1):k * (DH + 1) + DH],
                    pv[:, h * DH:(h + 1) * DH])

    def emit_a(blk):
        emit_xt_dma(blk)
        emit_kq(blk)
        emit_v(blk)

    def emit_c(qb, piece):
        """One out-projection piece (of NC_T*QH) for query block qb."""
        e4, j = divmod(piece, QH)
        jsl = slice(qb * QB + j * 512, qb * QB + (j + 1) * 512)
        o_ps = ps.tile([P, 512], F32, tag="s", name="o_ps")
        mm(o_ps[:], wo_sb[:, e4 * P:(e4 + 1) * P], ctx_sb[:, jsl])
        o_sb = osb.tile([P, 512], F32, tag="osb", name="o_sb")
        nc.vector.tensor_copy(o_sb[:], o_ps[:])
        nc.sync.dma_start(out=yt[e4 * P:(e4 + 1) * P, jsl], in_=o_sb[:])

    # phase A lead-in: K/Q for the first query block's tiles (V is injected
    # into the k-loop below - ctx matmuls need it ~3us later than scores)
    pre_blks = min(QH, NBLK)
    for blk in range(pre_blks):
        emit_xt_dma(blk)
    for blk in range(pre_blks):
        emit_kq(blk)

    NPIECE = NC_T * QH
    for qb in range(NQB):
        qsl = slice(qb * QB, (qb + 1) * QB)
        ctx_ps = [ps.tile([DH + 1, QB], F32, tag=f"ctx{h}", bufs=1,
                          name=f"ctx_ps{h}") for h in range(2)]
        pend = None  # (k, [e_h0, e_h1]) whose ctx matmuls are not yet emitted

        def emit_ctx(pk, pe):
            for h in range(2):
                vo = vones[h][:, pk * (DH + 1):(pk + 1) * (DH + 1)]
                for j in range(QH):
                    mm(ctx_ps[h][:, j * 512:(j + 1) * 512], vo,
                       pe[h][:, j * 512:(j + 1) * 512],
                       start=(pk == 0), stop=(pk == NK - 1))

        for k in range(NK):
            if qb == 0:
                # V blocks for the lead-in just ahead of their ctx use
                if k % 4 == 0 and k // 4 < pre_blks:
                    emit_v(k // 4)
                # remaining phase A blocks ~1.5 blocks ahead of use
                blk = (k + 6) // 4
                if pre_blks <= blk < NBLK and (k + 6) % 4 == 0:
                    emit_a(blk)
            # interleave previous query block's out-projection
            if qb > 0 and 4 <= k < 4 + NPIECE:
                emit_c(qb - 1, k - 4)
            s_ = [ps.tile([P, QB], F32, tag="s", name=f"s{h}") for h in range(2)]
            for j in range(QH):  # adjacent h issue => PE row-group packing
                jsl = slice(qb * QB + j * 512, qb * QB + (j + 1) * 512)
                for h in range(2):
                    hsl = slice(h * DH, (h + 1) * DH)
                    mm(s_[h][:, j * 512:(j + 1) * 512],
                       kt_sb[hsl, k * P:(k + 1) * P], qt_sb[hsl, jsl])
            e_ = []
            for h in range(2):
                e = ep.tile([P, QB], BF16, tag=f"e{h}", name=f"e{h}")
                nc.scalar.activation(e[:], s_[h][:],
                                     mybir.ActivationFunctionType.Exp,
                                     scale=inv_scale)
                e_.append(e)
            if pend is not None:
                emit_ctx(*pend)
            pend = (k, e_)
        emit_ctx(*pend)

        # normalize: drain accumulators first (frees PSUM in ~2.4us), then a
        # [128, QB/128]-reshaped reciprocal (DVE reciprocal is ~6 cycles per
        # FREE element per lane, so the [1, QB] row shape would cost 6.5us;
        # DMA-permuted to 128 partitions it costs ~50ns), then broadcast the
        # reciprocal row via a DRAM bounce and multiply.
        cds, rqs = [], []
        for h in range(2):
            cd = cdp.tile([DH + 1, QB], F32, tag=f"cd{h}", name=f"cd{h}")
            nc.vector.tensor_copy(cd[:], ctx_ps[h][:])
            cds.append(cd)
        for h in range(2):
            # denominator row -> [128, QB/128] (stream-order permutation)
            rq = bcp.tile([P, QB // P], F32, tag=f"rq{h}", name=f"rq{h}")
            nc.sync.dma_start(out=rq[:], in_=cds[h][DH:DH + 1, :])
            rqs.append(rq)
        for h in range(2):
            nc.vector.reciprocal(rqs[h][:], rqs[h][:])
            # back to a DRAM row (inverse of the same stream permutation)
            rtd = rdp.tile([1, QB], F32, tag="rtd", name="rtd")
            nc.sync.dma_start(out=rtd[:], in_=rqs[h][:])
            rtd_bcast = bass.AP(tensor=rtd.tensor, offset=rtd.offset,
                                ap=[[0, DH]] + list(rtd[0:1, :].ap)[1:])
            bc = bcp.tile([DH, QB], F32, tag=f"bc{h}", name=f"bc{h}")
            nc.sync.dma_start(out=bc[:], in_=rtd_bcast)
            hsl = slice(h * DH, (h + 1) * DH)
            nc.vector.tensor_mul(ctx_sb[hsl, qsl], cds[h][:DH, :], bc[:])

    # out-projection for the final query block
    for piece in range(NPIECE):
        emit_c(NQB - 1, piece)


_TPB_ENGINES = {mybir.EngineType.PE, mybir.EngineType.Activation,
                mybir.EngineType.DVE, mybir.EngineType.Pool}


def _legalize_matmul_waits(nc: bass.Bass) -> int:
    """Walrus encodes only ONE sync wait on TPB compute instructions (seen on
    Matmult and TensorCopy).  Move extra waits onto injected same-engine
    no-ops (one wait each) placed immediately before the instruction in its
    block: same semantics, legal encoding."""
    n_fixed = 0
    for f in nc.m.functions:
        for bb in f.blocks:
            out = []
            changed = False
            for ins in bb.instructions:
                si = ins.sync_info
                if (getattr(ins, "engine", None) is not None
                        and si is not None and len(si.on_wait) > 1):
                    for idx, w in enumerate(si.on_wait[:-1]):
                        nop = mybir.InstNoOp(name=f"{ins.name}-lgw{idx}",
                                             ins=[], outs=[])
                        nop.engine = ins.engine
                        nop.sync_info = mybir.SyncInfo(on_wait=[w], on_update=[])
                        out.append(nop)
                    ins.sync_info = mybir.SyncInfo(on_wait=[si.on_wait[-1]],
                                                   on_update=si.on_update)
                    n_fixed += 1
                    changed = True
                out.append(ins)
            if changed:
                bb.instructions = out
    return n_fixed


def build(S: int = S_FULL, legalize: bool = False) -> bass.Bass:
    nc = bass.Bass()
    with ExitStack() as ctx:
        ctx.enter_context(nc.allow_low_precision(
            reason="bf16 matmul operands / bf16 exp output"))
        tc = ctx.enter_context(tile.TileContext(nc))
        _emit(nc, tc, ctx, S)
    if legalize:
        # only for the walrus/hardware path; CoreSim wants updates on every
        # instruction and doesn't enforce the 1-wait Matmult limit
        _legalize_matmul_waits(nc)
    return nc


_NC_CACHE = {}


def _get_nc(S: int) -> bass.Bass:
    if S not in _NC_CACHE:
        _NC_CACHE[S] = build(S, legalize=True)
    return _NC_CACHE[S]


def make_in_maps(X, Wq, Wk, Wv, Wo):
    xts = [np.ascontiguousarray(X[b].T).astype(NP_BF16) for b in range(B)]
    in_maps = []
    for i in range(N_CORES):
        b, hp = divmod(i, 4)  # 4 head-pairs per batch
        csl = slice(hp * P, (hp + 1) * P)
        in_maps.append({
            "xt": xts[b],
            "wq": np.ascontiguousarray(Wq[:, csl]).astype(NP_BF16),
            "wk": np.ascontiguousarray(Wk[:, csl]).astype(NP_BF16),
            "wv": np.ascontiguousarray(Wv[:, csl]).astype(NP_BF16),
            "wo": np.ascontiguousarray(Wo[csl, :]).astype(NP_BF16),
        })
    return in_maps


def kernel(X, Wq, Wk, Wv, Wo, _trace=False):
    global LAST_RESULTS
    X = np.asarray(X, dtype=np.float32)
    S = X.shape[1]
    nc = _get_nc(S)
    in_maps = make_in_maps(X, np.asarray(Wq, np.float32), np.asarray(Wk, np.float32),
                           np.asarray(Wv, np.float32), np.asarray(Wo, np.float32))
    res = run_bass_kernel_spmd(nc, in_maps, list(range(N_CORES)), trace=_trace)
    LAST_RESULTS = res
    Y = np.zeros((B, S, D), dtype=np.float32)
    for i in range(N_CORES):
        Y[i // 4] += res.results[i]["yt"].T
    return Y


# revision 20
# speedup vs baseline: 1.6651x; 1.1026x over previous
"""Multi-head self-attention (B=2, S=4096, D=512, H=8, Dh=64) on 8 TRN2 cores.

Sharding: core i handles batch b = i//4 and head-pair hp = i%4 (heads 2*hp,
2*hp+1).  Each core computes Q/K/V projections for its two heads, flash-style
attention (no-max softmax; scores range is +-6 so exp is safe), and a partial
out-projection.  Host sums the 4 partial outputs per batch and transposes back.

v3 design notes (ACT-exp-bound pipeline):
- All matmul operands bf16 (fp16 moving operands stream at 2 cycles/col on the
  PE - measured 426ns for N=512 - while bf16 runs 1 col/cycle).
- Exp is the bottleneck engine (ACT: 1 elem/cycle/lane @1.2GHz + ~350cyc/call
  => 256 x 1147ns = 294us/core).  GPSIMD/Pool and DMA cannot read PSUM, so
  every scores element must leave PSUM through ACT or DVE; the ACT exp
  PSUM->SBUF(bf16) at [128, QB] granularity IS the optimal mover.
- Engine queues execute IN ORDER, so overlap is an emission-order problem:
  * ctx matmuls for k are emitted after the scores matmuls for k+1
    (software pipelining) - otherwise ctx(k), which waits on exp(k), blocks
    the already-runnable scores(k+1) in the PE queue and ACT starves.
  * projection blocks (phase A) are emitted interleaved into the first
    query-block's k-loop, just ahead of the k-tiles that consume them.
  * the out-projection for query-block qb is emitted in small pieces inside
    qb+1's k-loop, after the normalization data is long since ready.
- Normalization: ones-column appended to V gives the denominator row; the ctx
  accumulator is drained PSUM->SBUF by one DVE copy (freeing the PSUM bank in
  ~1.2us), the denominator row is partition-broadcast via a DRAM bounce
  (stride-0 partition APs are legal on DRAM), and the divide runs on the
  otherwise-idle GPSIMD/Pool engine, all off the critical path.
- PSUM: 2 x [128,QB] scores slots + 2 x [65,QB] ctx accumulators = 8 banks;
  phase A/C tiles rotate through the scores slots.
- ACT table prewarmed by a dummy exp at t=0 so the ~2.7us load overlaps DMA.

TRN2 quirk: walrus encodes only ONE sync wait on TPB compute instructions.
`_legalize_matmul_waits` post-processes the scheduled module: extra waits move
onto injected single-wait same-engine no-ops placed directly before the
instruction in its block - semantically identical, walrus-legal.
"""

import sys
from contextlib import ExitStack

for _p in ("/opt/trn_rl_repo",):
    if _p not in sys.path:
        sys.path.insert(0, _p)

import ml_dtypes
import numpy as np

import concourse.bass as bass
import concourse.tile as tile
from concourse import mybir
from concourse.bass_utils import run_bass_kernel_spmd

F32 = mybir.dt.float32
BF16 = mybir.dt.bfloat16
NP_BF16 = ml_dtypes.bfloat16
D = 512          # model dim
DH = 64          # head dim
P = 128          # partitions
B = 2
S_FULL = 4096
N_CORES = 8
NC_T = D // P    # 4 contraction tiles over model dim

LAST_RESULTS = None  # test harness reads exec_time_ns from here


def _emit(nc: bass.Bass, tc: "tile.TileContext", ctx: ExitStack, S: int):
    """Emit the per-core program. Parameterized by S for small-sim testing."""
    NBLK = S // 512          # 512-wide seq blocks
    NK = S // P              # 128-row key tiles
    QB = 1024 if S >= 1024 else S
    NQB = S // QB            # query blocks
    QH = QB // 512           # 512-wide halves per query block
    inv_scale = 1.0 / np.sqrt(DH)

    def mm(out, lhsT, rhs, start=True, stop=True):
        return nc.tensor.matmul(out, lhsT, rhs, start=start, stop=stop)

    xt = nc.declare_dram_parameter("xt", [D, S], BF16, isOutput=False)
    wq = nc.declare_dram_parameter("wq", [D, P], BF16, isOutput=False)
    wk = nc.declare_dram_parameter("wk", [D, P], BF16, isOutput=False)
    wv = nc.declare_dram_parameter("wv", [D, P], BF16, isOutput=False)
    wo = nc.declare_dram_parameter("wo", [P, D], BF16, isOutput=False)
    yt = nc.declare_dram_parameter("yt", [D, S], F32, isOutput=True)

    const = ctx.enter_context(tc.tile_pool(name="const", bufs=1))

    # ---- weights straight to SBUF ----
    w_sb = {}
    for name, ap in (("wq", wq), ("wk", wk), ("wv", wv)):
        tiles = []
        for c in range(NC_T):
            t = const.tile([P, P], BF16, tag=f"{name}{c}", name=f"{name}{c}")
            nc.sync.dma_start(out=t[:], in_=ap[c * P:(c + 1) * P, :])
            tiles.append(t)
        w_sb[name] = tiles
    wo_sb = const.tile([P, D], BF16, tag="wo")
    nc.sync.dma_start(out=wo_sb[:], in_=wo[:, :])

    # xt loaded in per-block chunks (interleaved with phase A below)
    xt_sb = [const.tile([P, S], BF16, tag=f"xt{c}", name=f"xt{c}")
             for c in range(NC_T)]

    # persistent intermediates
    qt_sb = const.tile([P, S], BF16, tag="qt")      # [2*64 d, S] stacked heads
    kt_sb = const.tile([P, S], BF16, tag="kt")
    # V with a ones column appended per k-tile: [128 k, NK*65]; col 64 == 1.0
    vones = [const.tile([P, NK * (DH + 1)], BF16, tag=f"vones{h}", name=f"vones{h}")
             for h in range(2)]
    konst = const.tile([P, max(NK, 2), 1], F32, tag="konst")
    nc.vector.memset(konst[:], 1.0)
    for h in range(2):
        vv = vones[h].rearrange("p (k c) -> p k c", c=DH + 1)
        nc.vector.tensor_copy(vv[:, :, DH:DH + 1], konst[:, :NK, :])
    ctx_sb = const.tile([P, S], BF16, tag="ctx")    # normalized context^T

    # ACT table prewarm: dummy exp so the ~2.7us table load overlaps phase A
    warm = const.tile([1, 2], F32, tag="warm")
    nc.scalar.activation(warm[:], konst[0:1, 0:2, 0], mybir.ActivationFunctionType.Exp)

    # PSUM: tag "s" 2x[128,QB] (2 banks each; also pq/pk/pv/o_ps) + 2 ctx accums
    ps = ctx.enter_context(tc.tile_pool(name="ps", bufs=2, space="PSUM"))
    ep = ctx.enter_context(tc.tile_pool(name="ep", bufs=3))
    bcp = ctx.enter_context(tc.tile_pool(name="bcp", bufs=2))
    cdp = ctx.enter_context(tc.tile_pool(name="cdp", bufs=2))
    rdp = ctx.enter_context(tc.tile_pool(name="rdp", bufs=2, space="DRAM"))
    osb = ctx.enter_context(tc.tile_pool(name="osb", bufs=2))

    def emit_xt_dma(blk):
        sl = slice(blk * 512, (blk + 1) * 512)
        for c in range(NC_T):
            nc.sync.dma_start(out=xt_sb[c][:, sl], in_=xt[c * P:(c + 1) * P, sl])

    def emit_kq(blk):
        """K and Q projections for one 512-wide block."""
        sl = slice(blk * 512, (blk + 1) * 512)
        pk = ps.tile([P, 512], F32, tag="s", name="pk")
        for c in range(NC_T):
            mm(pk[:], w_sb["wk"][c][:], xt_sb[c][:, sl],
               start=(c == 0), stop=(c == NC_T - 1))
        nc.vector.tensor_copy(kt_sb[:, sl], pk[:])
        pq = ps.tile([P, 512], F32, tag="s", name="pq")
        for c in range(NC_T):
            mm(pq[:], w_sb["wq"][c][:], xt_sb[c][:, sl],
               start=(c == 0), stop=(c == NC_T - 1))
        nc.vector.tensor_copy(qt_sb[:, sl], pq[:])

    def emit_v_tile(k):
        """V projection (+ones column layout) for one 128-key tile."""
        ksl = slice(k * P, (k + 1) * P)
        pv = ps.tile([P, P], F32, tag="s", name="pv")
        for c in range(NC_T):
            mm(pv[:], xt_sb[c][:, ksl], w_sb["wv"][c][:],
               start=(c == 0), stop=(c == NC_T - 1))
        for h in range(2):
            nc.vector.tensor_copy(
                vones[h][:, k * (DH + 1):k * (DH + 1) + DH],
                pv[:, h * DH:(h + 1) * DH])

    def emit_c(qb, piece):
        """One out-projection piece (of NC_T*QH) for query block qb."""
        e4, j = divmod(piece, QH)
        jsl = slice(qb * QB + j * 512, qb * QB + (j + 1) * 512)
        o_ps = ps.tile([P, 512], F32, tag="s", name="o_ps")
        mm(o_ps[:], wo_sb[:, e4 * P:(e4 + 1) * P], ctx_sb[:, jsl])
        o_sb = osb.tile([P, 512], F32, tag="osb", name="o_sb")
        nc.vector.tensor_copy(o_sb[:], o_ps[:])
        nc.sync.dma_start(out=yt[e4 * P:(e4 + 1) * P, jsl], in_=o_sb[:])

    # phase A lead-in: K/Q for the first query block's tiles.  The rest of
    # phase A is smeared over qb0's k-loop, ~one small item per k-iteration,
    # so the PE never gets a multi-us burst that starves the exp pipeline.
    pre_blks = min(QH, NBLK)
    for blk in range(min(pre_blks + 1, NBLK)):
        emit_xt_dma(blk)
    for blk in range(pre_blks):
        emit_kq(blk)
    inject = [[] for _ in range(NK)]
    for blk in range(pre_blks, NBLK):
        inject[max(0, 4 * blk - 8)].append(("kq", blk))
    for t in range(NK):
        inject[max(0, t - 2)].append(("v", t))

    NPIECE = NC_T * QH
    for qb in range(NQB):
        qsl = slice(qb * QB, (qb + 1) * QB)
        ctx_ps = [ps.tile([DH + 1, QB], F32, tag=f"ctx{h}", bufs=1,
                          name=f"ctx_ps{h}") for h in range(2)]
        pend = None  # (k, [e_h0, e_h1]) whose ctx matmuls are not yet emitted

        def emit_ctx(pk, pe):
            for h in range(2):
                vo = vones[h][:, pk * (DH + 1):(pk + 1) * (DH + 1)]
                for j in range(QH):
                    mm(ctx_ps[h][:, j * 512:(j + 1) * 512], vo,
                       pe[h][:, j * 512:(j + 1) * 512],
                       start=(pk == 0), stop=(pk == NK - 1))

        for k in range(NK):
            if qb == 0:
                for kind, idx in inject[k]:
                    if kind == "kq":
                        emit_kq(idx)
                        if idx + 1 < NBLK:
                            emit_xt_dma(idx + 1)
                    else:
                        emit_v_tile(idx)
            # interleave previous query block's out-projection (k >= 8 so the
            # ~11us normalize DMA-bounce latency is already paid)
            if qb > 0 and 8 <= k < 8 + NPIECE:
                emit_c(qb - 1, k - 8)
            s_ = [ps.tile([P, QB], F32, tag="s", name=f"s{h}") for h in range(2)]
            for h in range(2):
                hsl = slice(h * DH, (h + 1) * DH)
                for j in range(QH):
                    jsl = slice(qb * QB + j * 512, qb * QB + (j + 1) * 512)
                    mm(s_[h][:, j * 512:(j + 1) * 512],
                       kt_sb[hsl, k * P:(k + 1) * P], qt_sb[hsl, jsl])
            e_ = []
            for h in range(2):
                e = ep.tile([P, QB], BF16, tag=f"e{h}", name=f"e{h}")
                nc.scalar.activation(e[:], s_[h][:],
                                     mybir.ActivationFunctionType.Exp,
                                     scale=inv_scale)
                e_.append(e)
            if pend is not None:
                emit_ctx(*pend)
            pend = (k, e_)
        emit_ctx(*pend)

        # normalize: drain accumulators first (frees PSUM in ~2.4us), then a
        # [128, QB/128]-reshaped reciprocal (DVE reciprocal is ~6 cycles per
        # FREE element per lane, so the [1, QB] row shape would cost 6.5us;
        # DMA-permuted to 128 partitions it costs ~50ns), then broadcast the
        # reciprocal row via a DRAM bounce and multiply.
        cds, rqs = [], []
        for h in range(2):
            cd = cdp.tile([DH + 1, QB], F32, tag=f"cd{h}", name=f"cd{h}")
            nc.vector.tensor_copy(cd[:], ctx_ps[h][:])
            cds.append(cd)
        for h in range(2):
            # denominator row -> [128, QB/128] (stream-order permutation)
            rq = bcp.tile([P, QB // P], F32, tag=f"rq{h}", name=f"rq{h}")
            nc.sync.dma_start(out=rq[:], in_=cds[h][DH:DH + 1, :])
            rqs.append(rq)
        for h in range(2):
            nc.vector.reciprocal(rqs[h][:], rqs[h][:])
            # back to a DRAM row (inverse of the same stream permutation)
            rtd = rdp.tile([1, QB], F32, tag="rtd", name="rtd")
            nc.sync.dma_start(out=rtd[:], in_=rqs[h][:])
            rtd_bcast = bass.AP(tensor=rtd.tensor, offset=rtd.offset,
                                ap=[[0, DH]] + list(rtd[0:1, :].ap)[1:])
            bc = bcp.tile([DH, QB], F32, tag=f"bc{h}", name=f"bc{h}")
            nc.sync.dma_start(out=bc[:], in_=rtd_bcast)
            hsl = slice(h * DH, (h + 1) * DH)
            nc.vector.tensor_mul(ctx_sb[hsl, qsl], cds[h][:DH, :], bc[:])

    # out-projection for the final query block
    for piece in range(NPIECE):
        emit_c(NQB - 1, piece)


_TPB_ENGINES = {mybir.EngineType.PE, mybir.EngineType.Activation,
                mybir.EngineType.DVE, mybir.EngineType.Pool}


def _legalize_matmul_waits(nc: bass.Bass) -> int:
    """Walrus encodes only ONE sync wait on TPB compute instructions (seen on
    Matmult and TensorCopy).  Move extra waits onto injected same-engine
    no-ops (one wait each) placed immediately before the instruction in its
    block: same semantics, legal encoding."""
    n_fixed = 0
    for f in nc.m.functions:
        for bb in f.blocks:
            out = []
            changed = False
            for ins in bb.instructions:
                si = ins.sync_info
                if (getattr(ins, "engine", None) is not None
                        and si is not None and len(si.on_wait) > 1):
                    for idx, w in enumerate(si.on_wait[:-1]):
                        nop = mybir.InstNoOp(name=f"{ins.name}-lgw{idx}",
                                             ins=[], outs=[])
                        nop.engine = ins.engine
                        nop.sync_info = mybir.SyncInfo(on_wait=[w], on_update=[])
                        out.append(nop)
                    ins.sync_info = mybir.SyncInfo(on_wait=[si.on_wait[-1]],
                                                   on_update=si.on_update)
                    n_fixed += 1
                    changed = True
                out.append(ins)
            if changed:
                bb.instructions = out
    return n_fixed


def build(S: int = S_FULL, legalize: bool = False) -> bass.Bass:
    nc = bass.Bass()
    with ExitStack() as ctx:
        ctx.enter_context(nc.allow_low_precision(
            reason="bf16 matmul operands / bf16 exp output"))
        tc = ctx.enter_context(tile.TileContext(nc))
        _emit(nc, tc, ctx, S)
    if legalize:
        # only for the walrus/hardware path; CoreSim wants updates on every
        # instruction and doesn't enforce the 1-wait Matmult limit
        _legalize_matmul_waits(nc)
    return nc


_NC_CACHE = {}


def _get_nc(S: int) -> bass.Bass:
    if S not in _NC_CACHE:
        _NC_CACHE[S] = build(S, legalize=True)
    return _NC_CACHE[S]


def make_in_maps(X, Wq, Wk, Wv, Wo):
    xts = [np.ascontiguousarray(X[b].T).astype(NP_BF16) for b in range(B)]
    in_maps = []
    for i in range(N_CORES):
        b, hp = divmod(i, 4)  # 4 head-pairs per batch
        csl = slice(hp * P, (hp + 1) * P)
        in_maps.append({
            "xt": xts[b],
            "wq": np.ascontiguousarray(Wq[:, csl]).astype(NP_BF16),
            "wk": np.ascontiguousarray(Wk[:, csl]).astype(NP_BF16),
            "wv": np.ascontiguousarray(Wv[:, csl]).astype(NP_BF16),
            "wo": np.ascontiguousarray(Wo[csl, :]).astype(NP_BF16),
        })
    return in_maps


def kernel(X, Wq, Wk, Wv, Wo, _trace=False):
    global LAST_RESULTS
    X = np.asarray(X, dtype=np.float32)
    S = X.shape[1]
    nc = _get_nc(S)
    in_maps = make_in_maps(X, np.asarray(Wq, np.float32), np.asarray(Wk, np.float32),
                           np.asarray(Wv, np.float32), np.asarray(Wo, np.float32))
    res = run_bass_kernel_spmd(nc, in_maps, list(range(N_CORES)), trace=_trace)
    LAST_RESULTS = res
    Y = np.zeros((B, S, D), dtype=np.float32)
    for i in range(N_CORES):
        Y[i // 4] += res.results[i]["yt"].T
    return Y


# revision 24
# speedup vs baseline: 1.6787x; 1.0081x over previous
"""Multi-head self-attention (B=2, S=4096, D=512, H=8, Dh=64) on 8 TRN2 cores.

Sharding: core i handles batch b = i//4 and head-pair hp = i%4 (heads 2*hp,
2*hp+1).  Each core computes Q/K/V projections for its two heads, flash-style
attention (no-max softmax; scores range is +-6 so exp is safe), and a partial
out-projection.  Host sums the 4 partial outputs per batch and transposes back.

v3 design notes (ACT-exp-bound pipeline):
- All matmul operands bf16 (fp16 moving operands stream at 2 cycles/col on the
  PE - measured 426ns for N=512 - while bf16 runs 1 col/cycle).
- Exp is the bottleneck engine (ACT: 1 elem/cycle/lane @1.2GHz + ~350cyc/call
  => 256 x 1147ns = 294us/core).  GPSIMD/Pool and DMA cannot read PSUM, so
  every scores element must leave PSUM through ACT or DVE; the ACT exp
  PSUM->SBUF(bf16) at [128, QB] granularity IS the optimal mover.
- Engine queues execute IN ORDER, so overlap is an emission-order problem:
  * ctx matmuls for k are emitted after the scores matmuls for k+1
    (software pipelining) - otherwise ctx(k), which waits on exp(k), blocks
    the already-runnable scores(k+1) in the PE queue and ACT starves.
  * projection blocks (phase A) are emitted interleaved into the first
    query-block's k-loop, just ahead of the k-tiles that consume them.
  * the out-projection for query-block qb is emitted in small pieces inside
    qb+1's k-loop, after the normalization data is long since ready.
- Normalization: ones-column appended to V gives the denominator row; the ctx
  accumulator is drained PSUM->SBUF by one DVE copy (freeing the PSUM bank in
  ~1.2us), the denominator row is partition-broadcast via a DRAM bounce
  (stride-0 partition APs are legal on DRAM), and the divide runs on the
  otherwise-idle GPSIMD/Pool engine, all off the critical path.
- PSUM: 2 x [128,QB] scores slots + 2 x [65,QB] ctx accumulators = 8 banks;
  phase A/C tiles rotate through the scores slots.
- ACT table prewarmed by a dummy exp at t=0 so the ~2.7us load overlaps DMA.

TRN2 quirk: walrus encodes only ONE sync wait on TPB compute instructions.
`_legalize_matmul_waits` post-processes the scheduled module: extra waits move
onto injected single-wait same-engine no-ops placed directly before the
instruction in its block - semantically identical, walrus-legal.
"""

import sys
from contextlib import ExitStack

for _p in ("/opt/trn_rl_repo",):
    if _p not in sys.path:
        sys.path.insert(0, _p)

import ml_dtypes
import numpy as np

import concourse.bass as bass
import concourse.tile as tile
from concourse import mybir
from concourse.bass_utils import run_bass_kernel_spmd

F32 = mybir.dt.float32
BF16 = mybir.dt.bfloat16
F16 = mybir.dt.float16
NP_BF16 = ml_dtypes.bfloat16
D = 512          # model dim
DH = 64          # head dim
P = 128          # partitions
B = 2
S_FULL = 4096
N_CORES = 8
NC_T = D // P    # 4 contraction tiles over model dim

LAST_RESULTS = None  # test harness reads exec_time_ns from here


def _emit(nc: bass.Bass, tc: "tile.TileContext", ctx: ExitStack, S: int):
    """Emit the per-core program. Parameterized by S for small-sim testing."""
    NBLK = S // 512          # 512-wide seq blocks
    NK = S // P              # 128-row key tiles
    QB = 1024 if S >= 1024 else S
    NQB = S // QB            # query blocks
    QH = QB // 512           # 512-wide halves per query block
    inv_scale = 1.0 / np.sqrt(DH)

    def mm(out, lhsT, rhs, start=True, stop=True):
        return nc.tensor.matmul(out, lhsT, rhs, start=start, stop=stop)

    xt = nc.declare_dram_parameter("xt", [D, S], BF16, isOutput=False)
    wq = nc.declare_dram_parameter("wq", [D, P], BF16, isOutput=False)
    wk = nc.declare_dram_parameter("wk", [D, P], BF16, isOutput=False)
    wv = nc.declare_dram_parameter("wv", [D, P], BF16, isOutput=False)
    wo = nc.declare_dram_parameter("wo", [P, D], BF16, isOutput=False)
    yt = nc.declare_dram_parameter("yt", [D, S], F16, isOutput=True)

    const = ctx.enter_context(tc.tile_pool(name="const", bufs=1))

    # ---- weights straight to SBUF (wk first: the K projection runs first) --
    w_sb = {name: [const.tile([P, P], BF16, tag=f"{name}{c}", name=f"{name}{c}")
                   for c in range(NC_T)]
            for name in ("wq", "wk", "wv")}
    # xt loaded in per-block chunks (interleaved with phase A below)
    xt_sb = [const.tile([P, S], BF16, tag=f"xt{c}", name=f"xt{c}")
             for c in range(NC_T)]
    wo_sb = const.tile([P, D], BF16, tag="wo")

    def emit_w_dma(name, ap):
        for c in range(NC_T):
            nc.sync.dma_start(out=w_sb[name][c][:],
                              in_=ap[c * P:(c + 1) * P, :])

    # persistent intermediates
    qt_sb = const.tile([P, S], BF16, tag="qt")      # [2*64 d, S] stacked heads
    kt_sb = const.tile([P, S], BF16, tag="kt")
    # V with a ones column appended per k-tile: [128 k, NK*65]; col 64 == 1.0
    vones = [const.tile([P, NK * (DH + 1)], BF16, tag=f"vones{h}", name=f"vones{h}")
             for h in range(2)]
    konst = const.tile([P, max(NK, 2), 1], F32, tag="konst")
    nc.vector.memset(konst[:], 1.0)
    for h in range(2):
        vv = vones[h].rearrange("p (k c) -> p k c", c=DH + 1)
        nc.vector.tensor_copy(vv[:, :, DH:DH + 1], konst[:, :NK, :])
    ctx_sb = const.tile([P, S], BF16, tag="ctx")    # normalized context^T

    # ACT table prewarm: dummy exp so the ~2.7us table load overlaps phase A
    warm = const.tile([1, 2], F32, tag="warm")
    nc.scalar.activation(warm[:], konst[0:1, 0:2, 0], mybir.ActivationFunctionType.Exp)

    # PSUM: tag "s" 2x[128,QB] (2 banks each; also pq/pk/pv/o_ps) + 2 ctx accums
    ps = ctx.enter_context(tc.tile_pool(name="ps", bufs=2, space="PSUM"))
    ep = ctx.enter_context(tc.tile_pool(name="ep", bufs=3))
    bcp = ctx.enter_context(tc.tile_pool(name="bcp", bufs=2))
    cdp = ctx.enter_context(tc.tile_pool(name="cdp", bufs=2))
    rdp = ctx.enter_context(tc.tile_pool(name="rdp", bufs=2, space="DRAM"))
    osb = ctx.enter_context(tc.tile_pool(name="osb", bufs=2))

    def emit_xt_dma(blk):
        sl = slice(blk * 512, (blk + 1) * 512)
        for c in range(NC_T):
            nc.sync.dma_start(out=xt_sb[c][:, sl], in_=xt[c * P:(c + 1) * P, sl])

    def emit_kq(blk):
        """K and Q projections for one 512-wide block."""
        sl = slice(blk * 512, (blk + 1) * 512)
        pk = ps.tile([P, 512], F32, tag="s", name="pk")
        for c in range(NC_T):
            mm(pk[:], w_sb["wk"][c][:], xt_sb[c][:, sl],
               start=(c == 0), stop=(c == NC_T - 1))
        nc.vector.tensor_copy(kt_sb[:, sl], pk[:])
        pq = ps.tile([P, 512], F32, tag="s", name="pq")
        for c in range(NC_T):
            mm(pq[:], w_sb["wq"][c][:], xt_sb[c][:, sl],
               start=(c == 0), stop=(c == NC_T - 1))
        nc.vector.tensor_copy(qt_sb[:, sl], pq[:])

    def emit_v_tile(k):
        """V projection (+ones column layout) for one 128-key tile."""
        ksl = slice(k * P, (k + 1) * P)
        pv = ps.tile([P, P], F32, tag="s", name="pv")
        for c in range(NC_T):
            mm(pv[:], xt_sb[c][:, ksl], w_sb["wv"][c][:],
               start=(c == 0), stop=(c == NC_T - 1))
        for h in range(2):
            nc.vector.tensor_copy(
                vones[h][:, k * (DH + 1):k * (DH + 1) + DH],
                pv[:, h * DH:(h + 1) * DH])

    def emit_c(qb, piece):
        """One out-projection piece (of NC_T*QH) for query block qb."""
        e4, j = divmod(piece, QH)
        jsl = slice(qb * QB + j * 512, qb * QB + (j + 1) * 512)
        o_ps = ps.tile([P, 512], F32, tag="s", name="o_ps")
        mm(o_ps[:], wo_sb[:, e4 * P:(e4 + 1) * P], ctx_sb[:, jsl])
        o_sb = osb.tile([P, 512], F16, tag="osb", name="o_sb")
        nc.vector.tensor_copy(o_sb[:], o_ps[:])
        nc.sync.dma_start(out=yt[e4 * P:(e4 + 1) * P, jsl], in_=o_sb[:])

    # phase A lead-in: K/Q for the first query block's tiles.  The rest of
    # phase A is smeared over qb0's k-loop, ~one small item per k-iteration,
    # so the PE never gets a multi-us burst that starves the exp pipeline.
    # DMA priority order: wk + first xt block feed the very first matmul.
    pre_blks = min(QH, NBLK)
    emit_w_dma("wk", wk)
    emit_xt_dma(0)
    emit_w_dma("wq", wq)
    for blk in range(1, min(pre_blks + 1, NBLK)):
        emit_xt_dma(blk)
    emit_w_dma("wv", wv)
    nc.sync.dma_start(out=wo_sb[:], in_=wo[:, :])
    for blk in range(pre_blks):
        emit_kq(blk)
    inject = [[] for _ in range(NK)]
    for blk in range(pre_blks, NBLK):
        inject[max(0, 4 * blk - 8)].append(("kq", blk))
    for t in range(NK):
        inject[max(0, t - 2)].append(("v", t))

    NPIECE = NC_T * QH

    def emit_ctx(pqb, pk, pctx, pe):
        for h in range(2):
            vo = vones[h][:, pk * (DH + 1):(pk + 1) * (DH + 1)]
            for j in range(QH):
                mm(pctx[h][:, j * 512:(j + 1) * 512], vo,
                   pe[h][:, j * 512:(j + 1) * 512],
                   start=(pk == 0), stop=(pk == NK - 1))

    def emit_norm(pqb, pctx):
        # normalize: drain accumulators first (frees PSUM in ~2.4us), then a
        # [128, QB/128]-reshaped reciprocal (DVE reciprocal is ~6 cycles per
        # FREE element per lane, so the [1, QB] row shape would cost 6.5us;
        # DMA-permuted to 128 partitions it costs ~50ns), then broadcast the
        # reciprocal row via a DRAM bounce and multiply.
        qsl = slice(pqb * QB, (pqb + 1) * QB)
        cds, rqs = [], []
        for h in range(2):
            cd = cdp.tile([DH + 1, QB], F32, tag=f"cd{h}", name=f"cd{h}")
            nc.vector.tensor_copy(cd[:], pctx[h][:])
            cds.append(cd)
        for h in range(2):
            # denominator row -> [128, QB/128] (stream-order permutation)
            rq = bcp.tile([P, QB // P], F32, tag=f"rq{h}", name=f"rq{h}")
            nc.sync.dma_start(out=rq[:], in_=cds[h][DH:DH + 1, :])
            rqs.append(rq)
        for h in range(2):
            nc.vector.reciprocal(rqs[h][:], rqs[h][:])
            # back to a DRAM row (inverse of the same stream permutation)
            rtd = rdp.tile([1, QB], F32, tag="rtd", name="rtd")
            nc.sync.dma_start(out=rtd[:], in_=rqs[h][:])
            rtd_bcast = bass.AP(tensor=rtd.tensor, offset=rtd.offset,
                                ap=[[0, DH]] + list(rtd[0:1, :].ap)[1:])
            bc = bcp.tile([DH, QB], F32, tag=f"bc{h}", name=f"bc{h}")
            nc.sync.dma_start(out=bc[:], in_=rtd_bcast)
            hsl = slice(h * DH, (h + 1) * DH)
            nc.vector.tensor_mul(ctx_sb[hsl, qsl], cds[h][:DH, :], bc[:])

    # pend: (qb, k, ctx_ps_pair, e_pair) whose ctx matmuls are deferred until
    # after the NEXT k's scores - including across the qb boundary - so the
    # in-order PE queue never parks a not-yet-runnable ctx in front of
    # runnable scores.
    pend = None
    ctx_cur = None
    for qb in range(NQB):
        for k in range(NK):
            s_ = [ps.tile([P, QB], F32, tag="s", name=f"s{h}") for h in range(2)]
            for h in range(2):
                hsl = slice(h * DH, (h + 1) * DH)
                for j in range(QH):
                    jsl = slice(qb * QB + j * 512, qb * QB + (j + 1) * 512)
                    mm(s_[h][:, j * 512:(j + 1) * 512],
                       kt_sb[hsl, k * P:(k + 1) * P], qt_sb[hsl, jsl])
            e_ = []
            for h in range(2):
                e = ep.tile([P, QB], BF16, tag=f"e{h}", name=f"e{h}")
                nc.scalar.activation(e[:], s_[h][:],
                                     mybir.ActivationFunctionType.Exp,
                                     scale=inv_scale)
                e_.append(e)
            if pend is not None:
                emit_ctx(*pend)
                if pend[1] == NK - 1:  # previous query block fully accumulated
                    emit_norm(pend[0], pend[2])
            if k == 0:
                ctx_cur = [ps.tile([DH + 1, QB], F32, tag=f"ctx{h}", bufs=1,
                                   name=f"ctx_ps{h}") for h in range(2)]
            pend = (qb, k, ctx_cur, e_)
            # injections go AFTER this k's scores+exp so they consume PE slack
            # instead of delaying the exp feed
            if qb == 0:
                for kind, idx in inject[k]:
                    if kind == "kq":
                        emit_kq(idx)
                        if idx + 1 < NBLK:
                            emit_xt_dma(idx + 1)
                    else:
                        emit_v_tile(idx)
            # previous query block's out-projection (k >= 8: the ~11us
            # normalize DMA-bounce latency is already paid by then)
            if qb > 0 and 8 <= k < 8 + NPIECE:
                emit_c(qb - 1, k - 8)
    emit_ctx(*pend)
    emit_norm(pend[0], pend[2])

    # out-projection for the final query block
    for piece in range(NPIECE):
        emit_c(NQB - 1, piece)


_TPB_ENGINES = {mybir.EngineType.PE, mybir.EngineType.Activation,
                mybir.EngineType.DVE, mybir.EngineType.Pool}


def _legalize_matmul_waits(nc: bass.Bass) -> int:
    """Walrus encodes only ONE sync wait on TPB compute instructions (seen on
    Matmult and TensorCopy).  Move extra waits onto injected same-engine
    no-ops (one wait each) placed immediately before the instruction in its
    block: same semantics, legal encoding."""
    n_fixed = 0
    for f in nc.m.functions:
        for bb in f.blocks:
            out = []
            changed = False
            for ins in bb.instructions:
                si = ins.sync_info
                if (getattr(ins, "engine", None) is not None
                        and si is not None and len(si.on_wait) > 1):
                    for idx, w in enumerate(si.on_wait[:-1]):
                        nop = mybir.InstNoOp(name=f"{ins.name}-lgw{idx}",
                                             ins=[], outs=[])
                        nop.engine = ins.engine
                        nop.sync_info = mybir.SyncInfo(on_wait=[w], on_update=[])
                        out.append(nop)
                    ins.sync_info = mybir.SyncInfo(on_wait=[si.on_wait[-1]],
                                                   on_update=si.on_update)
                    n_fixed += 1
                    changed = True
                out.append(ins)
            if changed:
                bb.instructions = out
    return n_fixed


def build(S: int = S_FULL, legalize: bool = False) -> bass.Bass:
    nc = bass.Bass()
    with ExitStack() as ctx:
        ctx.enter_context(nc.allow_low_precision(
            reason="bf16 matmul operands / bf16 exp output"))
        tc = ctx.enter_context(tile.TileContext(nc))
        _emit(nc, tc, ctx, S)
    if legalize:
        # only for the walrus/hardware path; CoreSim wants updates on every
        # instruction and doesn't enforce the 1-wait Matmult limit
        _legalize_matmul_waits(nc)
    return nc


_NC_CACHE = {}


def _get_nc(S: int) -> bass.Bass:
    if S not in _NC_CACHE:
        _NC_CACHE[S] = build(S, legalize=True)
    return _NC_CACHE[S]


def make_in_maps(X, Wq, Wk, Wv, Wo):
    xts = [np.ascontiguousarray(X[b].T).astype(NP_BF16) for b in range(B)]
    in_maps = []
    for i in range(N_CORES):
        b, hp = divmod(i, 4)  # 4 head-pairs per batch
        csl = slice(hp * P, (hp + 1) * P)
        in_maps.append({
            "xt": xts[b],
            "wq": np.ascontiguousarray(Wq[:, csl]).astype(NP_BF16),
            "wk": np.ascontiguousarray(Wk[:, csl]).astype(NP_BF16),
            "wv": np.ascontiguousarray(Wv[:, csl]).astype(NP_BF16),
            "wo": np.ascontiguousarray(Wo[csl, :]).astype(NP_BF16),
        })
    return in_maps


def kernel(X, Wq, Wk, Wv, Wo, _trace=False):
    global LAST_RESULTS
    X = np.asarray(X, dtype=np.float32)
    S = X.shape[1]
    nc = _get_nc(S)
    in_maps = make_in_maps(X, np.asarray(Wq, np.float32), np.asarray(Wk, np.float32),
                           np.asarray(Wv, np.float32), np.asarray(Wo, np.float32))
    res = run_bass_kernel_spmd(nc, in_maps, list(range(N_CORES)), trace=_trace)
    LAST_RESULTS = res
    Y = np.zeros((B, S, D), dtype=np.float32)
    for i in range(N_CORES):
        Y[i // 4] += res.results[i]["yt"].T.astype(np.float32)
    return Y


# revision 27
# speedup vs baseline: 1.6803x; 1.0009x over previous
"""Multi-head self-attention (B=2, S=4096, D=512, H=8, Dh=64) on 8 TRN2 cores.

Sharding: core i handles batch b = i//4 and head-pair hp = i%4 (heads 2*hp,
2*hp+1).  Each core computes Q/K/V projections for its two heads, flash-style
attention (no-max softmax; scores range is +-6 so exp is safe), and a partial
out-projection.  Host sums the 4 partial outputs per batch and transposes back.

v3 design notes (ACT-exp-bound pipeline):
- All matmul operands bf16 (fp16 moving operands stream at 2 cycles/col on the
  PE - measured 426ns for N=512 - while bf16 runs 1 col/cycle).
- Exp is the bottleneck engine (ACT: 1 elem/cycle/lane @1.2GHz + ~350cyc/call
  => 256 x 1147ns = 294us/core).  GPSIMD/Pool and DMA cannot read PSUM, so
  every scores element must leave PSUM through ACT or DVE; the ACT exp
  PSUM->SBUF(bf16) at [128, QB] granularity IS the optimal mover.
- Engine queues execute IN ORDER, so overlap is an emission-order problem:
  * ctx matmuls for k are emitted after the scores matmuls for k+1
    (software pipelining) - otherwise ctx(k), which waits on exp(k), blocks
    the already-runnable scores(k+1) in the PE queue and ACT starves.
  * projection blocks (phase A) are emitted interleaved into the first
    query-block's k-loop, just ahead of the k-tiles that consume them.
  * the out-projection for query-block qb is emitted in small pieces inside
    qb+1's k-loop, after the normalization data is long since ready.
- Normalization: ones-column appended to V gives the denominator row; the ctx
  accumulator is drained PSUM->SBUF by one DVE copy (freeing the PSUM bank in
  ~1.2us), the denominator row is partition-broadcast via a DRAM bounce
  (stride-0 partition APs are legal on DRAM), and the divide runs on the
  otherwise-idle GPSIMD/Pool engine, all off the critical path.
- PSUM: 2 x [128,QB] scores slots + 2 x [65,QB] ctx accumulators = 8 banks;
  phase A/C tiles rotate through the scores slots.
- ACT table prewarmed by a dummy exp at t=0 so the ~2.7us load overlaps DMA.

TRN2 quirk: walrus encodes only ONE sync wait on TPB compute instructions.
`_legalize_matmul_waits` post-processes the scheduled module: extra waits move
onto injected single-wait same-engine no-ops placed directly before the
instruction in its block - semantically identical, walrus-legal.
"""

import sys
from contextlib import ExitStack

for _p in ("/opt/trn_rl_repo",):
    if _p not in sys.path:
        sys.path.insert(0, _p)

import ml_dtypes
import numpy as np

import concourse.bass as bass
import concourse.tile as tile
from concourse import mybir
from concourse.bass_utils import run_bass_kernel_spmd

F32 = mybir.dt.float32
BF16 = mybir.dt.bfloat16
F16 = mybir.dt.float16
NP_BF16 = ml_dtypes.bfloat16
D = 512          # model dim
DH = 64          # head dim
P = 128          # partitions
B = 2
S_FULL = 4096
N_CORES = 8
NC_T = D // P    # 4 contraction tiles over model dim

LAST_RESULTS = None  # test harness reads exec_time_ns from here


def _emit(nc: bass.Bass, tc: "tile.TileContext", ctx: ExitStack, S: int):
    """Emit the per-core program. Parameterized by S for small-sim testing."""
    NBLK = S // 512          # 512-wide seq blocks
    NK = S // P              # 128-row key tiles
    QB = 1024 if S >= 1024 else S
    NQB = S // QB            # query blocks
    QH = QB // 512           # 512-wide halves per query block
    inv_scale = 1.0 / np.sqrt(DH)

    def mm(out, lhsT, rhs, start=True, stop=True):
        return nc.tensor.matmul(out, lhsT, rhs, start=start, stop=stop)

    xt = nc.declare_dram_parameter("xt", [D, S], BF16, isOutput=False)
    wq = nc.declare_dram_parameter("wq", [D, P], BF16, isOutput=False)
    wk = nc.declare_dram_parameter("wk", [D, P], BF16, isOutput=False)
    wv = nc.declare_dram_parameter("wv", [D, P], BF16, isOutput=False)
    wo = nc.declare_dram_parameter("wo", [P, D], BF16, isOutput=False)
    yt = nc.declare_dram_parameter("yt", [D, S], F16, isOutput=True)

    const = ctx.enter_context(tc.tile_pool(name="const", bufs=1))

    # ---- weights straight to SBUF (wk first: the K projection runs first) --
    w_sb = {name: [const.tile([P, P], BF16, tag=f"{name}{c}", name=f"{name}{c}")
                   for c in range(NC_T)]
            for name in ("wq", "wk", "wv")}
    # xt loaded in per-block chunks (interleaved with phase A below)
    xt_sb = [const.tile([P, S], BF16, tag=f"xt{c}", name=f"xt{c}")
             for c in range(NC_T)]
    wo_sb = const.tile([P, D], BF16, tag="wo")

    def emit_w_dma(name, ap):
        for c in range(NC_T):
            nc.sync.dma_start(out=w_sb[name][c][:],
                              in_=ap[c * P:(c + 1) * P, :])

    # persistent intermediates
    qt_sb = const.tile([P, S], BF16, tag="qt")      # [2*64 d, S] stacked heads
    kt_sb = const.tile([P, S], BF16, tag="kt")
    # V with a ones column appended per k-tile: [128 k, NK*65]; col 64 == 1.0
    vones = [const.tile([P, NK * (DH + 1)], BF16, tag=f"vones{h}", name=f"vones{h}")
             for h in range(2)]
    konst = const.tile([P, max(NK, 2), 1], F32, tag="konst")
    nc.vector.memset(konst[:], 1.0)
    for h in range(2):
        vv = vones[h].rearrange("p (k c) -> p k c", c=DH + 1)
        nc.vector.tensor_copy(vv[:, :, DH:DH + 1], konst[:, :NK, :])
    ctx_sb = const.tile([P, S], BF16, tag="ctx")    # normalized context^T

    # ACT table prewarm: dummy exp so the ~2.7us table load overlaps phase A
    warm = const.tile([1, 2], F32, tag="warm")
    nc.scalar.activation(warm[:], konst[0:1, 0:2, 0], mybir.ActivationFunctionType.Exp)

    # PSUM: tag "s" 2x[128,QB] (2 banks each; also pq/pk/pv/o_ps) + 2 ctx accums
    ps = ctx.enter_context(tc.tile_pool(name="ps", bufs=2, space="PSUM"))
    ep = ctx.enter_context(tc.tile_pool(name="ep", bufs=3))
    bcp = ctx.enter_context(tc.tile_pool(name="bcp", bufs=2))
    cdp = ctx.enter_context(tc.tile_pool(name="cdp", bufs=2))
    rdp = ctx.enter_context(tc.tile_pool(name="rdp", bufs=2, space="DRAM"))
    osb = ctx.enter_context(tc.tile_pool(name="osb", bufs=2))

    def emit_xt_dma(blk):
        sl = slice(blk * 512, (blk + 1) * 512)
        for c in range(NC_T):
            nc.sync.dma_start(out=xt_sb[c][:, sl], in_=xt[c * P:(c + 1) * P, sl])

    def emit_k(blk):
        """K projection for one 512-wide block."""
        sl = slice(blk * 512, (blk + 1) * 512)
        pk = ps.tile([P, 512], F32, tag="s", name="pk")
        for c in range(NC_T):
            mm(pk[:], w_sb["wk"][c][:], xt_sb[c][:, sl],
               start=(c == 0), stop=(c == NC_T - 1))
        nc.vector.tensor_copy(kt_sb[:, sl], pk[:])

    def emit_q(blk):
        """Q projection for one 512-wide block."""
        sl = slice(blk * 512, (blk + 1) * 512)
        pq = ps.tile([P, 512], F32, tag="s", name="pq")
        for c in range(NC_T):
            mm(pq[:], w_sb["wq"][c][:], xt_sb[c][:, sl],
               start=(c == 0), stop=(c == NC_T - 1))
        nc.vector.tensor_copy(qt_sb[:, sl], pq[:])

    def emit_kq(blk):
        emit_k(blk)
        emit_q(blk)

    def emit_v_tile(k):
        """V projection (+ones column layout) for one 128-key tile."""
        ksl = slice(k * P, (k + 1) * P)
        pv = ps.tile([P, P], F32, tag="s", name="pv")
        for c in range(NC_T):
            mm(pv[:], xt_sb[c][:, ksl], w_sb["wv"][c][:],
               start=(c == 0), stop=(c == NC_T - 1))
        for h in range(2):
            nc.vector.tensor_copy(
                vones[h][:, k * (DH + 1):k * (DH + 1) + DH],
                pv[:, h * DH:(h + 1) * DH])

    def emit_c(qb, piece):
        """One out-projection piece (of NC_T*QH) for query block qb."""
        e4, j = divmod(piece, QH)
        jsl = slice(qb * QB + j * 512, qb * QB + (j + 1) * 512)
        o_ps = ps.tile([P, 512], F32, tag="s", name="o_ps")
        mm(o_ps[:], wo_sb[:, e4 * P:(e4 + 1) * P], ctx_sb[:, jsl])
        o_sb = osb.tile([P, 512], F16, tag="osb", name="o_sb")
        nc.vector.tensor_copy(o_sb[:], o_ps[:])
        nc.sync.dma_start(out=yt[e4 * P:(e4 + 1) * P, jsl], in_=o_sb[:])

    # phase A lead-in: K/Q for the first query block's tiles.  The rest of
    # phase A is smeared over qb0's k-loop, ~one small item per k-iteration,
    # so the PE never gets a multi-us burst that starves the exp pipeline.
    # DMA priority order: wk + first xt block feed the very first matmul.
    pre_blks = min(QH, NBLK)
    emit_w_dma("wk", wk)
    emit_xt_dma(0)
    emit_w_dma("wq", wq)
    for blk in range(1, min(pre_blks + 1, NBLK)):
        emit_xt_dma(blk)
    emit_w_dma("wv", wv)
    nc.sync.dma_start(out=wo_sb[:], in_=wo[:, :])
    for blk in range(pre_blks):
        emit_kq(blk)
    # injection schedule keyed (qb, k).  K projections and V tiles smear over
    # qb0 just ahead of their k-tile's use; the Q projection for blocks
    # feeding query block q is deferred into query block q-1's loop, where
    # the PE has slack (qb0 is the PE-tightest block).
    inject = {}
    for blk in range(pre_blks, NBLK):
        inject.setdefault((0, max(0, 4 * blk - 8)), []).append(("k", blk))
    for t in range(NK):
        inject.setdefault((0, max(0, t - 2)), []).append(("v", t))
    for blk in range(QH, NBLK):
        tgt = (blk // QH - 1, 16 + 4 * (blk % QH))
        inject.setdefault(tgt, []).append(("q", blk))

    NPIECE = NC_T * QH

    def emit_ctx(pqb, pk, pctx, pe):
        for h in range(2):
            vo = vones[h][:, pk * (DH + 1):(pk + 1) * (DH + 1)]
            for j in range(QH):
                mm(pctx[h][:, j * 512:(j + 1) * 512], vo,
                   pe[h][:, j * 512:(j + 1) * 512],
                   start=(pk == 0), stop=(pk == NK - 1))

    def emit_norm(pqb, pctx):
        # normalize: drain accumulators first (frees PSUM in ~2.4us), then a
        # [128, QB/128]-reshaped reciprocal (DVE reciprocal is ~6 cycles per
        # FREE element per lane, so the [1, QB] row shape would cost 6.5us;
        # DMA-permuted to 128 partitions it costs ~50ns), then broadcast the
        # reciprocal row via a DRAM bounce and multiply.
        qsl = slice(pqb * QB, (pqb + 1) * QB)
        cds, rqs = [], []
        for h in range(2):
            cd = cdp.tile([DH + 1, QB], F32, tag=f"cd{h}", name=f"cd{h}")
            nc.vector.tensor_copy(cd[:], pctx[h][:])
            cds.append(cd)
        for h in range(2):
            # denominator row -> [128, QB/128] (stream-order permutation)
            rq = bcp.tile([P, QB // P], F32, tag=f"rq{h}", name=f"rq{h}")
            nc.sync.dma_start(out=rq[:], in_=cds[h][DH:DH + 1, :])
            rqs.append(rq)
        for h in range(2):
            nc.vector.reciprocal(rqs[h][:], rqs[h][:])
            # back to a DRAM row (inverse of the same stream permutation)
            rtd = rdp.tile([1, QB], F32, tag="rtd", name="rtd")
            nc.sync.dma_start(out=rtd[:], in_=rqs[h][:])
            rtd_bcast = bass.AP(tensor=rtd.tensor, offset=rtd.offset,
                                ap=[[0, DH]] + list(rtd[0:1, :].ap)[1:])
            bc = bcp.tile([DH, QB], F32, tag=f"bc{h}", name=f"bc{h}")
            nc.sync.dma_start(out=bc[:], in_=rtd_bcast)
            hsl = slice(h * DH, (h + 1) * DH)
            nc.vector.tensor_mul(ctx_sb[hsl, qsl], cds[h][:DH, :], bc[:])

    # pend: (qb, k, ctx_ps_pair, e_pair) whose ctx matmuls are deferred until
    # after the NEXT k's scores - including across the qb boundary - so the
    # in-order PE queue never parks a not-yet-runnable ctx in front of
    # runnable scores.
    pend = None
    ctx_cur = None
    for qb in range(NQB):
        for k in range(NK):
            s_ = [ps.tile([P, QB], F32, tag="s", name=f"s{h}") for h in range(2)]
            for h in range(2):
                hsl = slice(h * DH, (h + 1) * DH)
                for j in range(QH):
                    jsl = slice(qb * QB + j * 512, qb * QB + (j + 1) * 512)
                    mm(s_[h][:, j * 512:(j + 1) * 512],
                       kt_sb[hsl, k * P:(k + 1) * P], qt_sb[hsl, jsl])
            e_ = []
            for h in range(2):
                e = ep.tile([P, QB], BF16, tag=f"e{h}", name=f"e{h}")
                nc.scalar.activation(e[:], s_[h][:],
                                     mybir.ActivationFunctionType.Exp,
                                     scale=inv_scale)
                e_.append(e)
            if pend is not None:
                emit_ctx(*pend)
                if pend[1] == NK - 1:  # previous query block fully accumulated
                    emit_norm(pend[0], pend[2])
            if k == 0:
                ctx_cur = [ps.tile([DH + 1, QB], F32, tag=f"ctx{h}", bufs=1,
                                   name=f"ctx_ps{h}") for h in range(2)]
            pend = (qb, k, ctx_cur, e_)
            # injections go AFTER this k's scores+exp so they consume PE slack
            # instead of delaying the exp feed
            for kind, idx in inject.get((qb, k), ()):
                if kind == "k":
                    emit_k(idx)
                    if idx + 1 < NBLK:
                        emit_xt_dma(idx + 1)
                elif kind == "q":
                    emit_q(idx)
                else:
                    emit_v_tile(idx)
            # previous query block's out-projection (k >= 8: the ~11us
            # normalize DMA-bounce latency is already paid by then)
            if qb > 0 and 8 <= k < 8 + NPIECE:
                emit_c(qb - 1, k - 8)
    emit_ctx(*pend)
    emit_norm(pend[0], pend[2])

    # out-projection for the final query block
    for piece in range(NPIECE):
        emit_c(NQB - 1, piece)


_TPB_ENGINES = {mybir.EngineType.PE, mybir.EngineType.Activation,
                mybir.EngineType.DVE, mybir.EngineType.Pool}


def _legalize_matmul_waits(nc: bass.Bass) -> int:
    """Walrus encodes only ONE sync wait on TPB compute instructions (seen on
    Matmult and TensorCopy).  Move extra waits onto injected same-engine
    no-ops (one wait each) placed immediately before the instruction in its
    block: same semantics, legal encoding."""
    n_fixed = 0
    for f in nc.m.functions:
        for bb in f.blocks:
            out = []
            changed = False
            for ins in bb.instructions:
                si = ins.sync_info
                if (getattr(ins, "engine", None) is not None
                        and si is not None and len(si.on_wait) > 1):
                    for idx, w in enumerate(si.on_wait[:-1]):
                        nop = mybir.InstNoOp(name=f"{ins.name}-lgw{idx}",
                                             ins=[], outs=[])
                        nop.engine = ins.engine
                        nop.sync_info = mybir.SyncInfo(on_wait=[w], on_update=[])
                        out.append(nop)
                    ins.sync_info = mybir.SyncInfo(on_wait=[si.on_wait[-1]],
                                                   on_update=si.on_update)
                    n_fixed += 1
                    changed = True
                out.append(ins)
            if changed:
                bb.instructions = out
    return n_fixed


def build(S: int = S_FULL, legalize: bool = False) -> bass.Bass:
    nc = bass.Bass()
    with ExitStack() as ctx:
        ctx.enter_context(nc.allow_low_precision(
            reason="bf16 matmul operands / bf16 exp output"))
        tc = ctx.enter_context(tile.TileContext(nc))
        _emit(nc, tc, ctx, S)
    if legalize:
        # only for the walrus/hardware path; CoreSim wants updates on every
        # instruction and doesn't enforce the 1-wait Matmult limit
        _legalize_matmul_waits(nc)
    return nc


_NC_CACHE = {}


def _get_nc(S: int) -> bass.Bass:
    if S not in _NC_CACHE:
        _NC_CACHE[S] = build(S, legalize=True)
    return _NC_CACHE[S]


def make_in_maps(X, Wq, Wk, Wv, Wo):
    xts = [np.ascontiguousarray(X[b].T).astype(NP_BF16) for b in range(B)]
    in_maps = []
    for i in range(N_CORES):
        b, hp = divmod(i, 4)  # 4 head-pairs per batch
        csl = slice(hp * P, (hp + 1) * P)
        in_maps.append({
            "xt": xts[b],
            "wq": np.ascontiguousarray(Wq[:, csl]).astype(NP_BF16),
            "wk": np.ascontiguousarray(Wk[:, csl]).astype(NP_BF16),
            "wv": np.ascontiguousarray(Wv[:, csl]).astype(NP_BF16),
            "wo": np.ascontiguousarray(Wo[csl, :]).astype(NP_BF16),
        })
    return in_maps


def kernel(X, Wq, Wk, Wv, Wo, _trace=False):
    global LAST_RESULTS
    X = np.asarray(X, dtype=np.float32)
    S = X.shape[1]
    nc = _get_nc(S)
    in_maps = make_in_maps(X, np.asarray(Wq, np.float32), np.asarray(Wk, np.float32),
                           np.asarray(Wv, np.float32), np.asarray(Wo, np.float32))
    res = run_bass_kernel_spmd(nc, in_maps, list(range(N_CORES)), trace=_trace)
    LAST_RESULTS = res
    Y = np.zeros((B, S, D), dtype=np.float32)
    for i in range(N_CORES):
        Y[i // 4] += res.results[i]["yt"].T.astype(np.float32)
    return Y
